# revision 8
# baseline (speedup 1.0000x reference)
"""BiARMA (2-layer ARMAConv GNN) Trainium2 kernel, 8-core SPMD — v2.

Changes vs v1:
  * Chunked AllGathers (4 slot-chunks) overlapped with edge-phase compute:
    block b of the gather reads AG chunk b, so C-phase block 0 starts as soon
    as the first chunk lands. Blocks are slot-chunks of all 8 cores
    (core-major within chunk) instead of core pairs.
  * Edge phase is block-outer / group-inner with f32 accumulation in SBUF
    (PSUM per (group, block), DVE add into acc).
  * x arrives host-pre-transposed [128, SHARD_PAD] and stays SBUF-resident;
    Phase A needs no PE transposes / DVE copies.
  * h tables are written only in cols 0:64 (the matmul never reads 64:128).
  * AllGather outputs are addr_space="Shared" (fast HBM-HBM path).
  * Tokens NOT sorted by source (random order measured faster on HW).
"""
import numpy as np
import ml_dtypes

N_CORES = 8
N_NODES = 100000
IN_CH, HID, CLS = 128, 64, 40
SHARD = 12500
SHARD_PAD = 12544          # 98 * 128
NT = SHARD_PAD // 128      # dest tiles per core (98)
VPAD = N_CORES * SHARD_PAD

# slot-chunks (AllGather chunks == gather blocks), in tiles
CHUNK_TILES = [25, 25, 25, 23]
NBLK = len(CHUNK_TILES)
CHUNK_ROWS = [t * 128 for t in CHUNK_TILES]            # per-core rows per chunk
CHUNK_START = np.concatenate([[0], np.cumsum(CHUNK_ROWS)])[:-1]
BLK_ROWS = [N_CORES * r for r in CHUNK_ROWS]           # table rows per block
BLK_START = np.concatenate([[0], np.cumsum(BLK_ROWS)])[:-1]
BLK_TILE_START = np.concatenate(
    [[0], np.cumsum([CHUNK_TILES[c] * N_CORES for c in range(NBLK)])])[:-1]
assert max(BLK_ROWS) <= 32767

# group split per chunk: groups are position-contiguous tile runs; psC needs
# G*64 f32 <= 2 PSUM banks -> G <= 16. Light/heavy caps experiment showed no
# viable c=2 tiles at this density, so caps are uniform.
LIGHT_PER_CHUNK = [0, 0, 0, 0]
GROUP_SPLIT = []          # list of (ntiles, cap_idx)
for _c in range(NBLK):
    L = LIGHT_PER_CHUNK[_c]
    H = CHUNK_TILES[_c] - L
    if L:
        GROUP_SPLIT.append((L, 2))
    # split heavy run into chunks of <=13 tiles (psC <= 832 f32 cols)
    while H > 0:
        take = min(13, H)
        GROUP_SPLIT.append((take, 3))
        H -= take
assert sum(g for g, _ in GROUP_SPLIT) == NT
NQUEUES = 4

bf16 = ml_dtypes.bfloat16
SUBCALL = 1024

_PROG_CACHE = {}


# ----------------------------------------------------------------------------
# host-side prep
# ----------------------------------------------------------------------------

def _pack_tiles(cnt, ntiles, dests, caps=None):
    """Greedy balanced packing of `dests` into `ntiles` tiles of 128 slots.

    cnt: [len(dests), K] per-dest edge counts by source block (K>=1).
    caps: optional [ntiles] per-block token capacity per tile; the greedy
    minimizes max-over-K of (sums+cnt)/cap (relative fill).
    Returns tile_of, slot_of (len(dests)).
    """
    tot = cnt.sum(1)
    order = np.argsort(-tot, kind="stable")
    K = cnt.shape[1]
    sums = np.zeros((ntiles, K), np.float64)
    nd = np.zeros(ntiles, np.int64)
    tile_of = np.empty(len(dests), np.int32)
    slot_of = np.empty(len(dests), np.int32)
    capv = np.ones(ntiles) if caps is None else np.asarray(caps, np.float64)
    BIG = 1 << 40
    for i in order:
        load = ((sums + cnt[i]) / capv[:, None]).max(axis=1) + (nd >= 128) * BIG
        t = int(np.argmin(load))
        tile_of[i] = t
        slot_of[i] = nd[t]
        nd[t] += 1
        sums[t] += cnt[i]
    return tile_of, slot_of


def _prep(edge_index):
    """Host prep. Returns per-core stream dict + c_tb."""
    row = np.asarray(edge_index[0]).astype(np.int64)
    col = np.asarray(edge_index[1]).astype(np.int64)
    deg = np.bincount(col, minlength=N_NODES).astype(np.float64)
    dinv = np.where(deg > 0, 1.0 / np.sqrt(np.maximum(deg, 1e-12)), 0.0).astype(np.float32)

    src_core = row // SHARD
    dst_core = col // SHARD
    chunk_of_slot = np.searchsorted(CHUNK_START, np.arange(SHARD_PAD),
                                    side="right") - 1

    # ---- pass 1: pack dests by TOTAL degree -> fixes each node's slot-CHUNK.
    # (An edge's gather block = its SOURCE node's slot-chunk; pass 2 only
    # moves dests between tiles of the same chunk, so blocks stay fixed.)
    deg_dst = np.bincount(col, minlength=N_NODES)
    chunk1 = []   # per core: chunk of each local dest [SHARD_PAD]
    for k in range(N_CORES):
        cnt = np.zeros((SHARD_PAD, 1), np.int64)
        cnt[:SHARD, 0] = deg_dst[k * SHARD:(k + 1) * SHARD]
        tile_of, _ = _pack_tiles(cnt, NT, np.arange(SHARD_PAD))
        chunk1.append(chunk_of_slot[tile_of * 128])

    # edge -> source chunk (fixed after pass 1)
    src_chunk = np.empty(len(row), np.int64)
    for k in range(N_CORES):
        m = src_core == k
        src_chunk[m] = chunk1[k][row[m] - k * SHARD]

    # ---- pass 2: within each chunk, re-pack dests by per-block counts,
    # with light tiles (first LIGHT_PER_CHUNK positions) on a smaller cap.
    packs = []
    for k in range(N_CORES):
        m = dst_core == k
        ec = col[m] - k * SHARD
        eb = src_chunk[m]
        cnt = np.zeros((SHARD_PAD, NBLK), np.int64)
        np.add.at(cnt, (ec, eb), 1)
        tile_of = np.empty(SHARD_PAD, np.int32)
        slot_of = np.empty(SHARD_PAD, np.int32)
        t0 = 0
        for c in range(NBLK):
            dests = np.nonzero(chunk1[k] == c)[0]
            assert len(dests) == CHUNK_TILES[c] * 128
            L = LIGHT_PER_CHUNK[c]
            caps = np.array([256.0] * L + [512.0] * (CHUNK_TILES[c] - L))
            tf, sf = _pack_tiles(cnt[dests], CHUNK_TILES[c], dests, caps=caps)
            tile_of[dests] = t0 + tf
            slot_of[dests] = sf
            t0 += CHUNK_TILES[c]
        packs.append((tile_of, slot_of))

    # global slot of each node (pass-2 slots; chunks unchanged from pass 1)
    slot_global = np.empty(N_NODES, np.int64)
    perms = []
    for k in range(N_CORES):
        tile_of, slot_of = packs[k]
        slot_idx = tile_of * 128 + slot_of
        slot_global[k * SHARD:(k + 1) * SHARD] = slot_idx[:SHARD]
        perm = np.full(SHARD_PAD, -1, np.int64)
        d_local = np.arange(SHARD_PAD)
        node = k * SHARD + d_local
        valid = d_local < SHARD
        perm[slot_idx[valid]] = node[valid]
        perms.append(perm)

    src_slot = slot_global[row]
    # table row within block: core-major
    src_row_in_blk = src_core * np.asarray(CHUNK_ROWS)[src_chunk] + (
        src_slot - CHUNK_START[src_chunk])
    assert (src_chunk == chunk_of_slot[src_slot]).all()

    # ---- pass 2: per-core streams
    cores = []
    for k in range(N_CORES):
        m = dst_core == k
        er_row_in_blk = src_row_in_blk[m]
        eb = src_chunk[m]
        ec = col[m] - k * SHARD
        tile_of, slot_of = packs[k]
        et = tile_of[ec]
        eslot = slot_of[ec]
        cnt = np.zeros((NT, NBLK), np.int64)
        np.add.at(cnt, (et, eb), 1)
        cores.append((er_row_in_blk, eb, et, eslot, cnt))

    # resolve per-(group, block) run length from actual max counts across cores
    maxcnt = np.zeros((NT, NBLK), np.int64)
    for c in cores:
        maxcnt = np.maximum(maxcnt, c[4])
    gsizes = [g for g, _ in GROUP_SPLIT]
    g_starts = np.concatenate([[0], np.cumsum(gsizes)])[:-1]
    groups = []            # resolved (ntiles, (cg per block))
    for gi, (G, _cap) in enumerate(GROUP_SPLIT):
        t0 = int(g_starts[gi])
        cgs = tuple(
            max(1, int(np.ceil(maxcnt[t0:t0 + G, b].max() / 128)))
            for b in range(NBLK))
        groups.append((G, cgs))
    groups = tuple(groups)
    TOT = sum(G * cg * 128 for G, cgs in groups for cg in cgs)

    # token stream layout: [g][b][tiles of g][cg_gb*128]
    base_bt = np.empty((NBLK, NT), np.int64)
    run_bt = np.empty((NBLK, NT), np.int64)
    off = 0
    for gi, (G, cgs) in enumerate(groups):
        for b in range(NBLK):
            run = cgs[b] * 128
            for tp in range(G):
                t = int(g_starts[gi]) + tp
                base_bt[b, t] = off
                run_bt[b, t] = run
                off += run
    assert off == TOT

    streams = []
    for k in range(N_CORES):
        er_blkrow, eb, et, eslot, _cnt = cores[k]
        key = eb * NT + et
        order = np.argsort(key, kind="stable")
        ks = key[order]
        grp_start = np.searchsorted(ks, ks)
        rank = np.arange(len(ks)) - grp_start
        pos = base_bt[eb[order], et[order]] + rank
        tok_src = np.zeros(TOT, np.int16)
        tok_colr = np.full(TOT, 200.0, np.float32)
        tok_src[pos] = er_blkrow[order].astype(np.int16)
        tok_colr[pos] = eslot[order].astype(np.float32)
        # layer-2 packed stream: same positions, packed row + half-selected colr
        r = er_blkrow[order]
        packed = (r // 128) * 64 + (r % 64)
        half = (r % 128) // 64
        tok_src2 = np.zeros(TOT, np.int16)
        tok_src2[pos] = packed.astype(np.int16)
        tok_colrA = np.full(TOT, 200.0, np.float32)
        tok_colrB = np.full(TOT, 200.0, np.float32)
        es = eslot[order].astype(np.float32)
        pA_ = pos[half == 0]
        pB_ = pos[half == 1]
        tok_colrA[pA_] = es[half == 0]
        tok_colrB[pB_] = es[half == 1]
        # idx16 wrap: [16, TOT/16], transposed per 16-token groups
        idx16 = tok_src.reshape(TOT // 16, 16).T.copy()
        idx128 = np.tile(idx16, (8, 1)).copy()
        idx16b = tok_src2.reshape(TOT // 16, 16).T.copy()
        idx128b = np.tile(idx16b, (8, 1)).copy()
        colr = tok_colr.reshape(TOT // 128, 128).T.astype(bf16).copy()
        colrA = tok_colrA.reshape(TOT // 128, 128).T.astype(bf16).copy()
        colrB = tok_colrB.reshape(TOT // 128, 128).T.astype(bf16).copy()
        # dinv in slot order [128, NT]
        perm = perms[k]
        dinv_slot = np.zeros(SHARD_PAD, np.float32)
        v = perm >= 0
        dinv_slot[v] = dinv[perm[v]]
        dinvT = dinv_slot.reshape(NT, 128).T.copy()
        streams.append(dict(idx=idx128, idx2=idx128b, colr=colr,
                            colrA=colrA, colrB=colrB, dinvT=dinvT, perm=perm))
    return streams, groups, TOT


# ----------------------------------------------------------------------------
# device program
# ----------------------------------------------------------------------------

def _build_program(groups, TOT):
    import concourse.bacc as bacc
    import concourse.mybir as mybir
    import concourse.tile as tile
    from concourse import library_config

    f32 = mybir.dt.float32
    b16 = mybir.dt.bfloat16
    i16 = mybir.dt.int16
    Copy = mybir.ActivationFunctionType.Copy
    Relu = mybir.ActivationFunctionType.Relu
    ADD = mybir.AluOpType.add
    MULT = mybir.AluOpType.mult
    ISEQ = mybir.AluOpType.is_equal

    nc = bacc.Bacc("TRN2", target_bir_lowering=False, debug=False,
                   num_devices=N_CORES, num_swdge_queues=NQUEUES)

    t_xT = nc.dram_tensor("xT", [128, SHARD_PAD], b16, kind="ExternalInput")
    t_xsb = nc.dram_tensor("xsb", [128, VPAD], b16, kind="ExternalInput")
    t_idx = nc.dram_tensor("idx", [128, TOT // 16], i16, kind="ExternalInput")
    t_idx2 = nc.dram_tensor("idx2", [128, TOT // 16], i16, kind="ExternalInput")
    t_colr = nc.dram_tensor("colr", [128, TOT // 128], b16, kind="ExternalInput")
    t_colrA = nc.dram_tensor("colrA", [128, TOT // 128], b16, kind="ExternalInput")
    t_colrB = nc.dram_tensor("colrB", [128, TOT // 128], b16, kind="ExternalInput")
    t_dinv = nc.dram_tensor("dinvT", [128, NT], f32, kind="ExternalInput")
    t_iota = nc.dram_tensor("iota", [128, 128], b16, kind="ExternalInput")
    t_ident = nc.dram_tensor("ident", [128, 128], f32, kind="ExternalInput")
    t_w1i = nc.dram_tensor("w1i", [128, 64], b16, kind="ExternalInput")
    t_w1r = nc.dram_tensor("w1r", [128, 64], b16, kind="ExternalInput")
    t_w2i = nc.dram_tensor("w2i", [64, 64], b16, kind="ExternalInput")
    t_w2r = nc.dram_tensor("w2r", [64, 64], b16, kind="ExternalInput")
    t_b1 = nc.dram_tensor("b1b", [128, 64], f32, kind="ExternalInput")
    t_b2 = nc.dram_tensor("b2b", [128, 64], f32, kind="ExternalInput")
    t_out = nc.dram_tensor("out", [SHARD_PAD, 64], f32, kind="ExternalOutput")

    CHUNK_W = max(G * cg * 128 for G, cg in groups)   # max tokens per (g,b)
    g_starts = np.concatenate([[0], np.cumsum([g for g, _ in groups])])[:-1]

    with tile.TileContext(nc) as tc:
        with (
            tc.tile_pool(name="cst", bufs=1) as cst,
            tc.tile_pool(name="acc", bufs=1) as accp,
            tc.tile_pool(name="hb", bufs=4) as hbp,
            tc.tile_pool(name="rootA", bufs=NT) as rootA,
            tc.tile_pool(name="rootB", bufs=NT) as rootB,
            tc.tile_pool(name="sp", bufs=6) as sp,
            tc.tile_pool(name="sxp", bufs=3) as sxp,
            tc.tile_pool(name="mp", bufs=3) as mp,
            tc.tile_pool(name="ohp", bufs=2) as ohp,
            tc.tile_pool(name="h1p", bufs=4) as h1p,
            tc.tile_pool(name="op", bufs=4) as op_,
            tc.tile_pool(name="psC", bufs=2, space="PSUM") as psC,
            tc.tile_pool(name="psA", bufs=2, space="PSUM") as psA,
            tc.tile_pool(name="psR", bufs=1, space="PSUM") as psR,
            tc.tile_pool(name="dram", bufs=1, space="DRAM") as dram,
        ):
            nc.gpsimd.load_library(library_config.mlp)

            def load_const(t, shape, dt, tag):
                s = cst.tile(shape, dt, tag=tag, name=tag)
                nc.sync.dma_start(s[:], t[:])
                return s

            iota_s = load_const(t_iota, [128, 128], b16, tag="iota_s")
            ident_s = load_const(t_ident, [128, 128], f32, tag="ident_s")
            w1i_s = load_const(t_w1i, [128, 64], b16, tag="w1i_s")
            w1r_s = load_const(t_w1r, [128, 64], b16, tag="w1r_s")
            w2i_s = load_const(t_w2i, [64, 64], b16, tag="w2i_s")
            w2r_s = load_const(t_w2r, [64, 64], b16, tag="w2r_s")
            b1_s = load_const(t_b1, [128, 64], f32, tag="b1_s")
            b2_s = load_const(t_b2, [128, 64], f32, tag="b2_s")
            dinv_s = load_const(t_dinv, [128, NT], f32, tag="dinv_s")
            xT_s = load_const(t_xT, [128, SHARD_PAD], b16, tag="xT_s")
            idx_s = load_const(t_idx, [128, TOT // 16], i16, tag="idx_s")
            idx2_s = load_const(t_idx2, [128, TOT // 16], i16, tag="idx2_s")
            colr_s = load_const(t_colr, [128, TOT // 128], b16, tag="colr_s")
            colrA_s = load_const(t_colrA, [128, TOT // 128], b16, tag="colrA_s")
            colrB_s = load_const(t_colrB, [128, TOT // 128], b16, tag="colrB_s")

            hh_own = dram.tile([SHARD_PAD // 2, 128], b16)
            h_full = [dram.tile([BLK_ROWS[c], 128], b16,
                                name=f"h_full_{c}") for c in range(NBLK)]
            hh_full = [dram.tile([BLK_ROWS[c] // 2, 128], b16,
                                 addr_space="Shared",
                                 name=f"hh_full_{c}") for c in range(NBLK)]

            # ------- Phase A: replicated full-table build (no AG1) -------
            # xsb is dinv-scaled full x, columns pre-permuted into table-row
            # order (block-major, core-major). Stream 16 tiles per DMA.
            STREAM = 16
            GW = 4      # tiles per grouped table write (block sizes % 4 == 0)
            NT_ALL = VPAD // 128
            assert all((CHUNK_TILES[c] * N_CORES) % GW == 0 for c in range(NBLK))
            pos = 0
            while pos < NT_ALL:
                take = min(STREAM, NT_ALL - pos)
                sx = sxp.tile([128, STREAM * 128], b16, tag="sx")
                nc.sync.dma_start(sx[:, 0:take * 128],
                                  t_xsb[:, pos * 128:(pos + take) * 128])
                for j0 in range(0, take, GW):
                    wide = hbp.tile([128, GW * 64], b16, tag="hbw")
                    for j in range(j0, j0 + GW):
                        t = pos + j
                        pA = psA.tile([128, 128], f32, tag="psA")
                        nc.tensor.matmul(out=pA[:, 0:64],
                                         lhsT=sx[:, j * 128:(j + 1) * 128],
                                         rhs=w1i_s[:], start=True, stop=True)
                        dst = wide[:, (j - j0) * 64:(j - j0 + 1) * 64]
                        nc.scalar.activation(dst, pA[:, 0:64], Copy)
                    t0g = pos + j0
                    c = int(np.searchsorted(BLK_TILE_START, t0g,
                                            side="right")) - 1
                    rr = (t0g - int(BLK_TILE_START[c])) * 128
                    nc.sync.dma_start(
                        h_full[c][rr:rr + GW * 128, 0:64].rearrange(
                            "(j p) f -> p j f", j=GW),
                        wide[:, :].rearrange("p (j f) -> p j f", j=GW))
                pos += take

            # ------- own-shard root1 (x @ w1_root + b1) -------
            root1 = []
            for t in range(NT):
                lhsT = xT_s[:, t * 128:(t + 1) * 128]
                pA = psA.tile([128, 128], f32, tag="psA")
                nc.tensor.matmul(out=pA[:, 64:128], lhsT=lhsT, rhs=w1r_s[:],
                                 start=True, stop=True)
                r1 = rootA.tile([128, 64], b16, tag="rootA")
                nc.vector.tensor_tensor(out=r1[:], in0=pA[:, 64:128],
                                        in1=b1_s[:], op=ADD)
                root1.append(r1)

            # -------- edge phase: group-outer, block-inner PSUM chains --------
            gsizes = [G for G, _ in groups]
            g_starts = np.concatenate([[0], np.cumsum(gsizes)])[:-1]
            CHUNK_W = max(G * cg * 128 for G, cgs in groups for cg in cgs)
            # absolute token offsets per (g, b), mirroring _prep layout
            off_gb = {}
            _off = 0
            for gi, (G, cgs) in enumerate(groups):
                for b in range(NBLK):
                    off_gb[(gi, b)] = _off
                    _off += G * cgs[b] * 128

            qstate = [0]

            def edge_group(table, gi, packed=False):
                G, cgs = groups[gi]
                acc = sp.tile([128, G * 64], f32, tag="eacc")
                my_idx = idx2_s if packed else idx_s
                for b in range(NBLK):
                    pC = psC.tile([128, G * 64], f32, tag="psC")
                    n = G * cgs[b] * 128
                    nch = G * cgs[b]
                    off = off_gb[(gi, b)]
                    jg = off // 128
                    m = mp.tile([128, CHUNK_W], b16, tag="mp")
                    done = 0
                    while done < n:
                        step = min(SUBCALL, n - done)
                        nc.gpsimd.dma_gather(
                            out_ap=m[:, done:done + step].rearrange(
                                "p (c d) -> p c d", d=128),
                            in_ap=table[b][:, :],
                            idxs_ap=my_idx[:, (off + done) // 16:
                                           (off + done + step) // 16],
                            num_idxs=step,
                            num_idxs_reg=step,
                            elem_size=128,
                            queue_num=qstate[0] % NQUEUES,
                        )
                        qstate[0] += 1
                        done += step
                    if not packed:
                        oh = ohp.tile([128, CHUNK_W], b16, tag="ohp")
                        nc.vector.tensor_tensor(
                            out=oh[:, :n].rearrange("p (c d) -> p c d", d=128),
                            in0=iota_s[:].unsqueeze(1).broadcast_to(
                                [128, nch, 128]),
                            in1=colr_s[:, jg:jg + nch].unsqueeze(2).broadcast_to(
                                [128, nch, 128]),
                            op=ISEQ,
                        )
                        for ti in range(G):
                            for cc in range(cgs[b]):
                                jj = (ti * cgs[b] + cc) * 128
                                nc.tensor.matmul(
                                    out=pC[:, ti * 64:(ti + 1) * 64],
                                    lhsT=oh[:, jj:jj + 128],
                                    rhs=m[:, jj:jj + 64],
                                    start=(cc == 0),
                                    stop=(cc == cgs[b] - 1),
                                )
                    else:
                        ohA = ohp.tile([128, CHUNK_W], b16, tag="ohp")
                        nc.vector.tensor_tensor(
                            out=ohA[:, :n].rearrange("p (c d) -> p c d", d=128),
                            in0=iota_s[:].unsqueeze(1).broadcast_to(
                                [128, nch, 128]),
                            in1=colrA_s[:, jg:jg + nch].unsqueeze(2).broadcast_to(
                                [128, nch, 128]),
                            op=ISEQ,
                        )
                        ohB = ohp.tile([128, CHUNK_W], b16, tag="ohp")
                        nc.vector.tensor_tensor(
                            out=ohB[:, :n].rearrange("p (c d) -> p c d", d=128),
                            in0=iota_s[:].unsqueeze(1).broadcast_to(
                                [128, nch, 128]),
                            in1=colrB_s[:, jg:jg + nch].unsqueeze(2).broadcast_to(
                                [128, nch, 128]),
                            op=ISEQ,
                        )
                        for ti in range(G):
                            for cc in range(cgs[b]):
                                jj = (ti * cgs[b] + cc) * 128
                                nc.tensor.matmul(
                                    out=pC[:, ti * 64:(ti + 1) * 64],
                                    lhsT=ohA[:, jj:jj + 128],
                                    rhs=m[:, jj:jj + 64],
                                    start=(cc == 0),
                                    stop=False,
                                )
                                nc.tensor.matmul(
                                    out=pC[:, ti * 64:(ti + 1) * 64],
                                    lhsT=ohB[:, jj:jj + 128],
                                    rhs=m[:, jj + 64:jj + 128],
                                    start=False,
                                    stop=(cc == cgs[b] - 1),
                                )
                    if b == 0:
                        nc.vector.tensor_copy(out=acc[:], in_=pC[:])
                    else:
                        nc.vector.tensor_tensor(out=acc[:], in0=acc[:],
                                                in1=pC[:], op=ADD)
                return acc

            groups_of_chunk = []
            _gi = 0
            for c in range(NBLK):
                tiles_left = CHUNK_TILES[c]
                lst = []
                while tiles_left > 0:
                    lst.append(_gi)
                    tiles_left -= groups[_gi][0]
                    _gi += 1
                groups_of_chunk.append(lst)

            # ---------------- C1 + D + chunked AG2 ----------------
            root2 = []
            for c in range(NBLK):
                for gi in groups_of_chunk[c]:
                    G = groups[gi][0]
                    pC = edge_group(h_full, gi)
                    for tp in range(G):
                        t = int(g_starts[gi]) + tp
                        s1 = sp.tile([128, 64], f32, tag="s1")
                        nc.scalar.activation(s1[:], pC[:, tp * 64:(tp + 1) * 64],
                                             Copy, scale=dinv_s[:, t:t + 1])
                        s2 = sp.tile([128, 64], f32, tag="s2")
                        nc.vector.tensor_tensor(out=s2[:], in0=s1[:],
                                                in1=root1[t][:], op=ADD)
                        pT = psR.tile([128, 128], f32, tag="psT")
                        nc.tensor.transpose(out=pT[:64, :], in_=s2[:],
                                            identity=ident_s[:])
                        h1t = h1p.tile([64, 128], b16, tag="h1t")
                        nc.scalar.activation(h1t[:], pT[:64, :], Relu)
                        pDB = psR.tile([128, 128], f32, tag="psDB")
                        nc.tensor.matmul(out=pDB[:, 0:64], lhsT=h1t[:],
                                         rhs=w2i_s[:], start=True, stop=True)
                        nc.tensor.matmul(out=pDB[:, 64:128], lhsT=h1t[:],
                                         rhs=w2r_s[:], start=True, stop=True)
                        hht = hbp.tile([128, 64], b16, tag="hb2")
                        nc.scalar.activation(hht[:], pDB[:, 0:64], Copy,
                                             scale=dinv_s[:, t:t + 1])
                        nc.sync.dma_start(
                            hh_own[t * 64:(t + 1) * 64, 0:64], hht[0:64, :])
                        nc.sync.dma_start(
                            hh_own[t * 64:(t + 1) * 64, 64:128], hht[64:128, :])
                        r2 = rootB.tile([128, 64], b16, tag="rootB")
                        nc.vector.tensor_tensor(out=r2[:], in0=pDB[:, 64:128],
                                                in1=b2_s[:], op=ADD)
                        root2.append(r2)
                r0 = int(CHUNK_START[c]) // 2
                r1_ = r0 + CHUNK_ROWS[c] // 2
                nc.gpsimd.collective_compute(
                    "AllGather", mybir.AluOpType.bypass,
                    replica_groups=[list(range(N_CORES))],
                    ins=[hh_own[r0:r1_, :].opt()], outs=[hh_full[c][:].opt()],
                )

            # ---------------- C2 -> output ----------------
            for gi in range(len(groups)):
                G = groups[gi][0]
                pC = edge_group(hh_full, gi, packed=True)
                for tp in range(G):
                    t = int(g_starts[gi]) + tp
                    s1 = sp.tile([128, 64], f32, tag="s1b")
                    nc.scalar.activation(s1[:], pC[:, tp * 64:(tp + 1) * 64],
                                         Copy, scale=dinv_s[:, t:t + 1])
                    s2 = sp.tile([128, 64], f32, tag="s2b")
                    nc.vector.tensor_tensor(out=s2[:], in0=s1[:],
                                            in1=root2[t][:], op=ADD)
                    o = op_.tile([128, 64], f32, tag="o")
                    nc.scalar.activation(o[:], s2[:], Relu)
                    nc.sync.dma_start(t_out[t * 128:(t + 1) * 128, :], o[:])

    nc.compile()
    return nc


# ----------------------------------------------------------------------------
# entry point
# ----------------------------------------------------------------------------

_LAST_RESULTS = None


def build_in_maps(inputs, streams):
    x = np.asarray(inputs["x"], np.float32)
    iota = np.broadcast_to(np.arange(128, dtype=np.float32), (128, 128)).astype(bf16)
    ident = np.eye(128, dtype=np.float32)
    w1i = np.asarray(inputs["w1_init"], np.float32)
    w1r = np.asarray(inputs["w1_root"], np.float32)
    w2i = np.zeros((64, 64), np.float32); w2i[:, :CLS] = inputs["w2_init"]
    w2r = np.zeros((64, 64), np.float32); w2r[:, :CLS] = inputs["w2_root"]
    b1b = np.broadcast_to(np.asarray(inputs["b1"], np.float32), (128, HID)).copy()
    b2p = np.zeros(64, np.float32); b2p[:CLS] = inputs["b2"]
    b2b = np.broadcast_to(b2p, (128, 64)).copy()
    xs_all = np.zeros((VPAD, 128), np.float32)
    col = 0
    for c in range(NBLK):
        for k in range(N_CORES):
            s0 = int(CHUNK_START[c]); nrow = CHUNK_ROWS[c]
            slots = np.arange(s0, s0 + nrow)
            perm = streams[k]["perm"]
            nodes = perm[slots]
            v = nodes >= 0
            dv = streams[k]["dinvT"].T.reshape(-1)
            blk = np.zeros((nrow, 128), np.float32)
            blk[v] = x[nodes[v]] * dv[slots[v], None]
            xs_all[col:col + nrow] = blk
            col += nrow
    xsb = xs_all.T.astype(bf16).copy()
    in_maps = []
    for k in range(N_CORES):
        s = streams[k]
        perm = s["perm"]
        xk = np.zeros((SHARD_PAD, 128), np.float32)
        v = perm >= 0
        xk[v] = x[perm[v]]
        in_maps.append(dict(
            xT=xk.T.astype(bf16).copy(),
            xsb=xsb,
            idx=s["idx"], idx2=s["idx2"], colr=np.asarray(s["colr"]),
            colrA=np.asarray(s["colrA"]), colrB=np.asarray(s["colrB"]),
            dinvT=s["dinvT"],
            iota=np.asarray(iota), ident=ident,
            w1i=w1i.astype(bf16), w1r=w1r.astype(bf16),
            w2i=w2i.astype(bf16), w2r=w2r.astype(bf16),
            b1b=b1b, b2b=b2b,
        ))
    return in_maps


def kernel(x, edge_index, w1_init, w1_root, b1, w2_init, w2_root, b2, **kw):
    global _LAST_RESULTS
    from concourse.bass_utils import run_bass_kernel_spmd

    inputs = dict(x=x, edge_index=edge_index, w1_init=w1_init, w1_root=w1_root,
                  b1=b1, w2_init=w2_init, w2_root=w2_root, b2=b2)
    streams, groups, TOT = _prep(np.asarray(edge_index))

    key = (groups, TOT)
    if key not in _PROG_CACHE:
        _PROG_CACHE[key] = _build_program(groups, TOT)
    nc = _PROG_CACHE[key]

    in_maps = build_in_maps(inputs, streams)

    import os
    trace = os.environ.get("BIARMA_TRACE", "0") == "1"
    res = run_bass_kernel_spmd(nc, in_maps, core_ids=list(range(N_CORES)),
                               trace=trace)
    _LAST_RESULTS = res

    out = np.zeros((N_NODES, CLS), np.float32)
    for k in range(N_CORES):
        o = res.results[k]["out"]
        perm = streams[k]["perm"]
        v = perm >= 0
        out[perm[v]] = o[v][:, :CLS]
    return out



# revision 10
# speedup vs baseline: 2.0435x; 2.0435x over previous
"""BiARMA (2-layer ARMAConv GNN) Trainium2 kernel, 8-core SPMD — v2.

Changes vs v1:
  * Chunked AllGathers (4 slot-chunks) overlapped with edge-phase compute:
    block b of the gather reads AG chunk b, so C-phase block 0 starts as soon
    as the first chunk lands. Blocks are slot-chunks of all 8 cores
    (core-major within chunk) instead of core pairs.
  * Edge phase is block-outer / group-inner with f32 accumulation in SBUF
    (PSUM per (group, block), DVE add into acc).
  * x arrives host-pre-transposed [128, SHARD_PAD] and stays SBUF-resident;
    Phase A needs no PE transposes / DVE copies.
  * h tables are written only in cols 0:64 (the matmul never reads 64:128).
  * AllGather outputs are addr_space="Shared" (fast HBM-HBM path).
  * Tokens NOT sorted by source (random order measured faster on HW).
"""
import numpy as np
import ml_dtypes

N_CORES = 8
N_NODES = 100000
IN_CH, HID, CLS = 128, 64, 40
SHARD = 12500
SHARD_PAD = 12544          # 98 * 128
NT = SHARD_PAD // 128      # dest tiles per core (98)
VPAD = N_CORES * SHARD_PAD

# slot-chunks (AllGather chunks == gather blocks), in tiles
CHUNK_TILES = [25, 25, 25, 23]
NBLK = len(CHUNK_TILES)
CHUNK_ROWS = [t * 128 for t in CHUNK_TILES]            # per-core rows per chunk
CHUNK_START = np.concatenate([[0], np.cumsum(CHUNK_ROWS)])[:-1]
BLK_ROWS = [N_CORES * r for r in CHUNK_ROWS]           # table rows per block
BLK_START = np.concatenate([[0], np.cumsum(BLK_ROWS)])[:-1]
BLK_TILE_START = np.concatenate(
    [[0], np.cumsum([CHUNK_TILES[c] * N_CORES for c in range(NBLK)])])[:-1]
assert max(BLK_ROWS) <= 32767

# group split per chunk: groups are position-contiguous tile runs; psC needs
# G*64 f32 <= 2 PSUM banks -> G <= 16. Light/heavy caps experiment showed no
# viable c=2 tiles at this density, so caps are uniform.
LIGHT_PER_CHUNK = [0, 0, 0, 0]
GROUP_SPLIT = []          # list of (ntiles, cap_idx)
for _c in range(NBLK):
    L = LIGHT_PER_CHUNK[_c]
    H = CHUNK_TILES[_c] - L
    if L:
        GROUP_SPLIT.append((L, 2))
    # split heavy run into chunks of <=13 tiles (psC <= 832 f32 cols)
    while H > 0:
        take = min(13, H)
        GROUP_SPLIT.append((take, 3))
        H -= take
assert sum(g for g, _ in GROUP_SPLIT) == NT
NQUEUES = 4

bf16 = ml_dtypes.bfloat16
SUBCALL = 1024

_PROG_CACHE = {}


# ----------------------------------------------------------------------------
# host-side prep
# ----------------------------------------------------------------------------

def _pack_tiles(cnt, ntiles, dests, caps=None):
    """Greedy balanced packing of `dests` into `ntiles` tiles of 128 slots.

    cnt: [len(dests), K] per-dest edge counts by source block (K>=1).
    caps: optional [ntiles] per-block token capacity per tile; the greedy
    minimizes max-over-K of (sums+cnt)/cap (relative fill).
    Returns tile_of, slot_of (len(dests)).
    """
    tot = cnt.sum(1)
    order = np.argsort(-tot, kind="stable")
    K = cnt.shape[1]
    sums = np.zeros((ntiles, K), np.float64)
    nd = np.zeros(ntiles, np.int64)
    tile_of = np.empty(len(dests), np.int32)
    slot_of = np.empty(len(dests), np.int32)
    capv = np.ones(ntiles) if caps is None else np.asarray(caps, np.float64)
    BIG = 1 << 40
    for i in order:
        load = ((sums + cnt[i]) / capv[:, None]).max(axis=1) + (nd >= 128) * BIG
        t = int(np.argmin(load))
        tile_of[i] = t
        slot_of[i] = nd[t]
        nd[t] += 1
        sums[t] += cnt[i]
    return tile_of, slot_of


def _prep(edge_index):
    """Host prep. Returns per-core stream dict + c_tb."""
    row = np.asarray(edge_index[0]).astype(np.int64)
    col = np.asarray(edge_index[1]).astype(np.int64)
    deg = np.bincount(col, minlength=N_NODES).astype(np.float64)
    dinv = np.where(deg > 0, 1.0 / np.sqrt(np.maximum(deg, 1e-12)), 0.0).astype(np.float32)

    src_core = row // SHARD
    dst_core = col // SHARD
    chunk_of_slot = np.searchsorted(CHUNK_START, np.arange(SHARD_PAD),
                                    side="right") - 1

    # ---- pass 1: pack dests by TOTAL degree -> fixes each node's slot-CHUNK.
    # (An edge's gather block = its SOURCE node's slot-chunk; pass 2 only
    # moves dests between tiles of the same chunk, so blocks stay fixed.)
    deg_dst = np.bincount(col, minlength=N_NODES)
    chunk1 = []   # per core: chunk of each local dest [SHARD_PAD]
    for k in range(N_CORES):
        cnt = np.zeros((SHARD_PAD, 1), np.int64)
        cnt[:SHARD, 0] = deg_dst[k * SHARD:(k + 1) * SHARD]
        tile_of, _ = _pack_tiles(cnt, NT, np.arange(SHARD_PAD))
        chunk1.append(chunk_of_slot[tile_of * 128])

    # edge -> source chunk (fixed after pass 1)
    src_chunk = np.empty(len(row), np.int64)
    for k in range(N_CORES):
        m = src_core == k
        src_chunk[m] = chunk1[k][row[m] - k * SHARD]

    # ---- pass 2: within each chunk, re-pack dests by per-block counts,
    # with light tiles (first LIGHT_PER_CHUNK positions) on a smaller cap.
    packs = []
    for k in range(N_CORES):
        m = dst_core == k
        ec = col[m] - k * SHARD
        eb = src_chunk[m]
        cnt = np.zeros((SHARD_PAD, NBLK), np.int64)
        np.add.at(cnt, (ec, eb), 1)
        tile_of = np.empty(SHARD_PAD, np.int32)
        slot_of = np.empty(SHARD_PAD, np.int32)
        t0 = 0
        for c in range(NBLK):
            dests = np.nonzero(chunk1[k] == c)[0]
            assert len(dests) == CHUNK_TILES[c] * 128
            L = LIGHT_PER_CHUNK[c]
            caps = np.array([256.0] * L + [512.0] * (CHUNK_TILES[c] - L))
            tf, sf = _pack_tiles(cnt[dests], CHUNK_TILES[c], dests, caps=caps)
            tile_of[dests] = t0 + tf
            slot_of[dests] = sf
            t0 += CHUNK_TILES[c]
        packs.append((tile_of, slot_of))

    # global slot of each node (pass-2 slots; chunks unchanged from pass 1)
    slot_global = np.empty(N_NODES, np.int64)
    perms = []
    for k in range(N_CORES):
        tile_of, slot_of = packs[k]
        slot_idx = tile_of * 128 + slot_of
        slot_global[k * SHARD:(k + 1) * SHARD] = slot_idx[:SHARD]
        perm = np.full(SHARD_PAD, -1, np.int64)
        d_local = np.arange(SHARD_PAD)
        node = k * SHARD + d_local
        valid = d_local < SHARD
        perm[slot_idx[valid]] = node[valid]
        perms.append(perm)

    src_slot = slot_global[row]
    # table row within block: core-major
    src_row_in_blk = src_core * np.asarray(CHUNK_ROWS)[src_chunk] + (
        src_slot - CHUNK_START[src_chunk])
    assert (src_chunk == chunk_of_slot[src_slot]).all()

    # ---- pass 2: per-core streams
    cores = []
    for k in range(N_CORES):
        m = dst_core == k
        er_row_in_blk = src_row_in_blk[m]
        eb = src_chunk[m]
        ec = col[m] - k * SHARD
        tile_of, slot_of = packs[k]
        et = tile_of[ec]
        eslot = slot_of[ec]
        cnt = np.zeros((NT, NBLK), np.int64)
        np.add.at(cnt, (et, eb), 1)
        cores.append((er_row_in_blk, eb, et, eslot, cnt))

    # resolve per-(group, block) run length from actual max counts across cores
    maxcnt = np.zeros((NT, NBLK), np.int64)
    for c in cores:
        maxcnt = np.maximum(maxcnt, c[4])
    gsizes = [g for g, _ in GROUP_SPLIT]
    g_starts = np.concatenate([[0], np.cumsum(gsizes)])[:-1]
    groups = []            # resolved (ntiles, (cg per block))
    for gi, (G, _cap) in enumerate(GROUP_SPLIT):
        t0 = int(g_starts[gi])
        cgs = tuple(
            max(1, int(np.ceil(maxcnt[t0:t0 + G, b].max() / 128)))
            for b in range(NBLK))
        groups.append((G, cgs))
    groups = tuple(groups)
    TOT = sum(G * cg * 128 for G, cgs in groups for cg in cgs)

    # token stream layout: [g][b][tiles of g][cg_gb*128]
    base_bt = np.empty((NBLK, NT), np.int64)
    run_bt = np.empty((NBLK, NT), np.int64)
    off = 0
    for gi, (G, cgs) in enumerate(groups):
        for b in range(NBLK):
            run = cgs[b] * 128
            for tp in range(G):
                t = int(g_starts[gi]) + tp
                base_bt[b, t] = off
                run_bt[b, t] = run
                off += run
    assert off == TOT

    streams = []
    for k in range(N_CORES):
        er_blkrow, eb, et, eslot, _cnt = cores[k]
        key = eb * NT + et
        order = np.argsort(key, kind="stable")
        ks = key[order]
        grp_start = np.searchsorted(ks, ks)
        rank = np.arange(len(ks)) - grp_start
        pos = base_bt[eb[order], et[order]] + rank
        tok_src = np.zeros(TOT, np.int16)
        tok_colr = np.full(TOT, 200.0, np.float32)
        tok_src[pos] = er_blkrow[order].astype(np.int16)
        tok_colr[pos] = eslot[order].astype(np.float32)
        # layer-2 packed stream: same positions, packed row + half-selected colr
        r = er_blkrow[order]
        packed = (r // 128) * 64 + (r % 64)
        half = (r % 128) // 64
        tok_src2 = np.zeros(TOT, np.int16)
        tok_src2[pos] = packed.astype(np.int16)
        tok_colrA = np.full(TOT, 200.0, np.float32)
        tok_colrB = np.full(TOT, 200.0, np.float32)
        es = eslot[order].astype(np.float32)
        pA_ = pos[half == 0]
        pB_ = pos[half == 1]
        tok_colrA[pA_] = es[half == 0]
        tok_colrB[pB_] = es[half == 1]
        # idx16 wrap: [16, TOT/16], transposed per 16-token groups
        idx16 = tok_src.reshape(TOT // 16, 16).T.copy()
        idx128 = np.tile(idx16, (8, 1)).copy()
        idx16b = tok_src2.reshape(TOT // 16, 16).T.copy()
        idx128b = np.tile(idx16b, (8, 1)).copy()
        colr = tok_colr.reshape(TOT // 128, 128).T.astype(bf16).copy()
        colrA = tok_colrA.reshape(TOT // 128, 128).T.astype(bf16).copy()
        colrB = tok_colrB.reshape(TOT // 128, 128).T.astype(bf16).copy()
        # dinv in slot order [128, NT]
        perm = perms[k]
        dinv_slot = np.zeros(SHARD_PAD, np.float32)
        v = perm >= 0
        dinv_slot[v] = dinv[perm[v]]
        dinvT = dinv_slot.reshape(NT, 128).T.copy()
        streams.append(dict(idx=idx128, idx2=idx128b, colr=colr,
                            colrA=colrA, colrB=colrB, dinvT=dinvT, perm=perm))
    return streams, groups, TOT


# ----------------------------------------------------------------------------
# device program
# ----------------------------------------------------------------------------

def _build_program(groups, TOT):
    import concourse.bacc as bacc
    import concourse.mybir as mybir
    import concourse.tile as tile
    from concourse import library_config

    f32 = mybir.dt.float32
    b16 = mybir.dt.bfloat16
    i16 = mybir.dt.int16
    Copy = mybir.ActivationFunctionType.Copy
    Relu = mybir.ActivationFunctionType.Relu
    ADD = mybir.AluOpType.add
    MULT = mybir.AluOpType.mult
    ISEQ = mybir.AluOpType.is_equal

    nc = bacc.Bacc("TRN2", target_bir_lowering=False, debug=False,
                   num_devices=N_CORES, num_swdge_queues=NQUEUES)

    t_xT = nc.dram_tensor("xT", [128, SHARD_PAD], b16, kind="ExternalInput")
    t_xsb = nc.dram_tensor("xsb", [128, VPAD], b16, kind="ExternalInput")
    t_idx = nc.dram_tensor("idx", [128, TOT // 16], i16, kind="ExternalInput")
    t_idx2 = nc.dram_tensor("idx2", [128, TOT // 16], i16, kind="ExternalInput")
    t_colr = nc.dram_tensor("colr", [128, TOT // 128], b16, kind="ExternalInput")
    t_colrA = nc.dram_tensor("colrA", [128, TOT // 128], b16, kind="ExternalInput")
    t_colrB = nc.dram_tensor("colrB", [128, TOT // 128], b16, kind="ExternalInput")
    t_dinv = nc.dram_tensor("dinvT", [128, NT], f32, kind="ExternalInput")
    t_iota = nc.dram_tensor("iota", [128, 128], b16, kind="ExternalInput")
    t_ident = nc.dram_tensor("ident", [128, 128], f32, kind="ExternalInput")
    t_w1i = nc.dram_tensor("w1i", [128, 64], b16, kind="ExternalInput")
    t_w1r = nc.dram_tensor("w1r", [128, 64], b16, kind="ExternalInput")
    t_w2i = nc.dram_tensor("w2i", [64, 64], b16, kind="ExternalInput")
    t_w2r = nc.dram_tensor("w2r", [64, 64], b16, kind="ExternalInput")
    t_b1 = nc.dram_tensor("b1b", [128, 64], f32, kind="ExternalInput")
    t_b2 = nc.dram_tensor("b2b", [128, 64], f32, kind="ExternalInput")
    t_out = nc.dram_tensor("out", [SHARD_PAD, 64], f32, kind="ExternalOutput")

    CHUNK_W = max(G * cg * 128 for G, cg in groups)   # max tokens per (g,b)
    g_starts = np.concatenate([[0], np.cumsum([g for g, _ in groups])])[:-1]

    with tile.TileContext(nc) as tc:
        with (
            tc.tile_pool(name="cst", bufs=1) as cst,
            tc.tile_pool(name="acc", bufs=1) as accp,
            tc.tile_pool(name="hb", bufs=4) as hbp,
            tc.tile_pool(name="rootA", bufs=NT) as rootA,
            tc.tile_pool(name="rootB", bufs=NT) as rootB,
            tc.tile_pool(name="sp", bufs=6) as sp,
            tc.tile_pool(name="sxp", bufs=3) as sxp,
            tc.tile_pool(name="mp", bufs=3) as mp,
            tc.tile_pool(name="ohp", bufs=2) as ohp,
            tc.tile_pool(name="h1p", bufs=4) as h1p,
            tc.tile_pool(name="op", bufs=4) as op_,
            tc.tile_pool(name="psC", bufs=2, space="PSUM") as psC,
            tc.tile_pool(name="psA", bufs=2, space="PSUM") as psA,
            tc.tile_pool(name="psR", bufs=1, space="PSUM") as psR,
            tc.tile_pool(name="dram", bufs=1, space="DRAM") as dram,
        ):
            nc.gpsimd.load_library(library_config.mlp)

            def load_const(t, shape, dt, tag):
                s = cst.tile(shape, dt, tag=tag, name=tag)
                nc.sync.dma_start(s[:], t[:])
                return s

            iota_s = load_const(t_iota, [128, 128], b16, tag="iota_s")
            ident_s = load_const(t_ident, [128, 128], f32, tag="ident_s")
            w1i_s = load_const(t_w1i, [128, 64], b16, tag="w1i_s")
            w1r_s = load_const(t_w1r, [128, 64], b16, tag="w1r_s")
            w2i_s = load_const(t_w2i, [64, 64], b16, tag="w2i_s")
            w2r_s = load_const(t_w2r, [64, 64], b16, tag="w2r_s")
            b1_s = load_const(t_b1, [128, 64], f32, tag="b1_s")
            b2_s = load_const(t_b2, [128, 64], f32, tag="b2_s")
            dinv_s = load_const(t_dinv, [128, NT], f32, tag="dinv_s")
            xT_s = load_const(t_xT, [128, SHARD_PAD], b16, tag="xT_s")
            idx_s = load_const(t_idx, [128, TOT // 16], i16, tag="idx_s")
            idx2_s = load_const(t_idx2, [128, TOT // 16], i16, tag="idx2_s")
            colr_s = load_const(t_colr, [128, TOT // 128], b16, tag="colr_s")
            colrA_s = load_const(t_colrA, [128, TOT // 128], b16, tag="colrA_s")
            colrB_s = load_const(t_colrB, [128, TOT // 128], b16, tag="colrB_s")

            hh_own = dram.tile([SHARD_PAD // 2, 128], b16)
            h_full = [dram.tile([BLK_ROWS[c], 128], b16,
                                name=f"h_full_{c}") for c in range(NBLK)]
            hh_full = [dram.tile([BLK_ROWS[c] // 2, 128], b16,
                                 addr_space="Shared",
                                 name=f"hh_full_{c}") for c in range(NBLK)]

            # ------- Phase A: replicated full-table build (no AG1) -------
            # xsb is dinv-scaled full x, columns pre-permuted into table-row
            # order (block-major, core-major). Stream 16 tiles per DMA.
            STREAM = 16
            GW = 4      # tiles per grouped table write (block sizes % 4 == 0)
            NT_ALL = VPAD // 128
            assert all((CHUNK_TILES[c] * N_CORES) % GW == 0 for c in range(NBLK))
            pos = 0
            while pos < NT_ALL:
                take = min(STREAM, NT_ALL - pos)
                sx = sxp.tile([128, STREAM * 128], b16, tag="sx")
                nc.sync.dma_start(sx[:, 0:take * 128],
                                  t_xsb[:, pos * 128:(pos + take) * 128])
                for j0 in range(0, take, GW):
                    wide = hbp.tile([128, GW * 64], b16, tag="hbw")
                    pA4 = psA.tile([128, GW * 64], f32, tag="psA4")
                    for j in range(j0, j0 + GW):
                        nc.tensor.matmul(out=pA4[:, (j - j0) * 64:
                                                  (j - j0 + 1) * 64],
                                         lhsT=sx[:, j * 128:(j + 1) * 128],
                                         rhs=w1i_s[:], start=True, stop=True)
                    nc.scalar.activation(wide[:, :], pA4[:, :], Copy)
                    t0g = pos + j0
                    c = int(np.searchsorted(BLK_TILE_START, t0g,
                                            side="right")) - 1
                    rr = (t0g - int(BLK_TILE_START[c])) * 128
                    nc.sync.dma_start(
                        h_full[c][rr:rr + GW * 128, 0:64].rearrange(
                            "(j p) f -> p j f", j=GW),
                        wide[:, :].rearrange("p (j f) -> p j f", j=GW))
                pos += take

            # ------- own-shard root1 (x @ w1_root + b1) -------
            root1 = []
            for t in range(NT):
                lhsT = xT_s[:, t * 128:(t + 1) * 128]
                pA = psA.tile([128, GW * 64], f32, tag="psA4")
                nc.tensor.matmul(out=pA[:, 64:128], lhsT=lhsT, rhs=w1r_s[:],
                                 start=True, stop=True)
                r1 = rootA.tile([128, 64], b16, tag="rootA")
                nc.vector.tensor_tensor(out=r1[:], in0=pA[:, 64:128],
                                        in1=b1_s[:], op=ADD)
                root1.append(r1)

            # -------- edge phase: group-outer, block-inner PSUM chains --------
            gsizes = [G for G, _ in groups]
            g_starts = np.concatenate([[0], np.cumsum(gsizes)])[:-1]
            CHUNK_W = max(G * cg * 128 for G, cgs in groups for cg in cgs)
            # absolute token offsets per (g, b), mirroring _prep layout
            off_gb = {}
            _off = 0
            for gi, (G, cgs) in enumerate(groups):
                for b in range(NBLK):
                    off_gb[(gi, b)] = _off
                    _off += G * cgs[b] * 128

            qstate = [0]

            def edge_group(table, gi, packed=False):
                G, cgs = groups[gi]
                acc = sp.tile([128, G * 64], f32, tag="eacc")
                my_idx = idx2_s if packed else idx_s
                for b in range(NBLK):
                    pC = psC.tile([128, G * 64], f32, tag="psC")
                    n = G * cgs[b] * 128
                    nch = G * cgs[b]
                    off = off_gb[(gi, b)]
                    jg = off // 128
                    m = mp.tile([128, CHUNK_W], b16, tag="mp")
                    done = 0
                    while done < n:
                        step = min(SUBCALL, n - done)
                        nc.gpsimd.dma_gather(
                            out_ap=m[:, done:done + step].rearrange(
                                "p (c d) -> p c d", d=128),
                            in_ap=table[b][:, :],
                            idxs_ap=my_idx[:, (off + done) // 16:
                                           (off + done + step) // 16],
                            num_idxs=step,
                            num_idxs_reg=step,
                            elem_size=128,
                            queue_num=qstate[0] % NQUEUES,
                        )
                        qstate[0] += 1
                        done += step
                    if not packed:
                        oh = ohp.tile([128, CHUNK_W], b16, tag="ohp")
                        nc.vector.tensor_tensor(
                            out=oh[:, :n].rearrange("p (c d) -> p c d", d=128),
                            in0=iota_s[:].unsqueeze(1).broadcast_to(
                                [128, nch, 128]),
                            in1=colr_s[:, jg:jg + nch].unsqueeze(2).broadcast_to(
                                [128, nch, 128]),
                            op=ISEQ,
                        )
                        for ti in range(G):
                            for cc in range(cgs[b]):
                                jj = (ti * cgs[b] + cc) * 128
                                nc.tensor.matmul(
                                    out=pC[:, ti * 64:(ti + 1) * 64],
                                    lhsT=oh[:, jj:jj + 128],
                                    rhs=m[:, jj:jj + 64],
                                    start=(cc == 0),
                                    stop=(cc == cgs[b] - 1),
                                )
                    else:
                        ohA = ohp.tile([128, CHUNK_W], b16, tag="ohp")
                        nc.vector.tensor_tensor(
                            out=ohA[:, :n].rearrange("p (c d) -> p c d", d=128),
                            in0=iota_s[:].unsqueeze(1).broadcast_to(
                                [128, nch, 128]),
                            in1=colrA_s[:, jg:jg + nch].unsqueeze(2).broadcast_to(
                                [128, nch, 128]),
                            op=ISEQ,
                        )
                        ohB = ohp.tile([128, CHUNK_W], b16, tag="ohp")
                        nc.vector.tensor_tensor(
                            out=ohB[:, :n].rearrange("p (c d) -> p c d", d=128),
                            in0=iota_s[:].unsqueeze(1).broadcast_to(
                                [128, nch, 128]),
                            in1=colrB_s[:, jg:jg + nch].unsqueeze(2).broadcast_to(
                                [128, nch, 128]),
                            op=ISEQ,
                        )
                        for ti in range(G):
                            for cc in range(cgs[b]):
                                jj = (ti * cgs[b] + cc) * 128
                                nc.tensor.matmul(
                                    out=pC[:, ti * 64:(ti + 1) * 64],
                                    lhsT=ohA[:, jj:jj + 128],
                                    rhs=m[:, jj:jj + 64],
                                    start=(cc == 0),
                                    stop=False,
                                )
                                nc.tensor.matmul(
                                    out=pC[:, ti * 64:(ti + 1) * 64],
                                    lhsT=ohB[:, jj:jj + 128],
                                    rhs=m[:, jj + 64:jj + 128],
                                    start=False,
                                    stop=(cc == cgs[b] - 1),
                                )
                    if b == 0:
                        nc.vector.tensor_copy(out=acc[:], in_=pC[:])
                    else:
                        nc.vector.tensor_tensor(out=acc[:], in0=acc[:],
                                                in1=pC[:], op=ADD)
                return acc

            groups_of_chunk = []
            _gi = 0
            for c in range(NBLK):
                tiles_left = CHUNK_TILES[c]
                lst = []
                while tiles_left > 0:
                    lst.append(_gi)
                    tiles_left -= groups[_gi][0]
                    _gi += 1
                groups_of_chunk.append(lst)

            # ---------------- C1 + D + chunked AG2 ----------------
            root2 = []
            for c in range(NBLK):
                for gi in groups_of_chunk[c]:
                    G = groups[gi][0]
                    pC = edge_group(h_full, gi)
                    for tp in range(G):
                        t = int(g_starts[gi]) + tp
                        s1 = sp.tile([128, 64], f32, tag="s1")
                        nc.scalar.activation(s1[:], pC[:, tp * 64:(tp + 1) * 64],
                                             Copy, scale=dinv_s[:, t:t + 1])
                        s2 = sp.tile([128, 64], f32, tag="s2")
                        nc.vector.tensor_tensor(out=s2[:], in0=s1[:],
                                                in1=root1[t][:], op=ADD)
                        pT = psR.tile([128, 128], f32, tag="psT")
                        nc.tensor.transpose(out=pT[:64, :], in_=s2[:],
                                            identity=ident_s[:])
                        h1t = h1p.tile([64, 128], b16, tag="h1t")
                        nc.scalar.activation(h1t[:], pT[:64, :], Relu)
                        pDB = psR.tile([128, 128], f32, tag="psDB")
                        nc.tensor.matmul(out=pDB[:, 0:64], lhsT=h1t[:],
                                         rhs=w2i_s[:], start=True, stop=True)
                        nc.tensor.matmul(out=pDB[:, 64:128], lhsT=h1t[:],
                                         rhs=w2r_s[:], start=True, stop=True)
                        hht = hbp.tile([128, 64], b16, tag="hb2")
                        nc.scalar.activation(hht[:], pDB[:, 0:64], Copy,
                                             scale=dinv_s[:, t:t + 1])
                        nc.sync.dma_start(
                            hh_own[t * 64:(t + 1) * 64, 0:64], hht[0:64, :])
                        nc.sync.dma_start(
                            hh_own[t * 64:(t + 1) * 64, 64:128], hht[64:128, :])
                        r2 = rootB.tile([128, 64], b16, tag="rootB")
                        nc.vector.tensor_tensor(out=r2[:], in0=pDB[:, 64:128],
                                                in1=b2_s[:], op=ADD)
                        root2.append(r2)
                r0 = int(CHUNK_START[c]) // 2
                r1_ = r0 + CHUNK_ROWS[c] // 2
                nc.gpsimd.collective_compute(
                    "AllGather", mybir.AluOpType.bypass,
                    replica_groups=[list(range(N_CORES))],
                    ins=[hh_own[r0:r1_, :].opt()], outs=[hh_full[c][:].opt()],
                )

            # ---------------- C2 -> output ----------------
            for gi in range(len(groups)):
                G = groups[gi][0]
                pC = edge_group(hh_full, gi, packed=True)
                for tp in range(G):
                    t = int(g_starts[gi]) + tp
                    s1 = sp.tile([128, 64], f32, tag="s1b")
                    nc.scalar.activation(s1[:], pC[:, tp * 64:(tp + 1) * 64],
                                         Copy, scale=dinv_s[:, t:t + 1])
                    s2 = sp.tile([128, 64], f32, tag="s2b")
                    nc.vector.tensor_tensor(out=s2[:], in0=s1[:],
                                            in1=root2[t][:], op=ADD)
                    o = op_.tile([128, 64], f32, tag="o")
                    nc.scalar.activation(o[:], s2[:], Relu)
                    nc.sync.dma_start(t_out[t * 128:(t + 1) * 128, :], o[:])

    nc.compile()
    return nc


# ----------------------------------------------------------------------------
# entry point
# ----------------------------------------------------------------------------

_LAST_RESULTS = None


def build_in_maps(inputs, streams):
    x = np.asarray(inputs["x"], np.float32)
    iota = np.broadcast_to(np.arange(128, dtype=np.float32), (128, 128)).astype(bf16)
    ident = np.eye(128, dtype=np.float32)
    w1i = np.asarray(inputs["w1_init"], np.float32)
    w1r = np.asarray(inputs["w1_root"], np.float32)
    w2i = np.zeros((64, 64), np.float32); w2i[:, :CLS] = inputs["w2_init"]
    w2r = np.zeros((64, 64), np.float32); w2r[:, :CLS] = inputs["w2_root"]
    b1b = np.broadcast_to(np.asarray(inputs["b1"], np.float32), (128, HID)).copy()
    b2p = np.zeros(64, np.float32); b2p[:CLS] = inputs["b2"]
    b2b = np.broadcast_to(b2p, (128, 64)).copy()
    xs_all = np.zeros((VPAD, 128), np.float32)
    col = 0
    for c in range(NBLK):
        for k in range(N_CORES):
            s0 = int(CHUNK_START[c]); nrow = CHUNK_ROWS[c]
            slots = np.arange(s0, s0 + nrow)
            perm = streams[k]["perm"]
            nodes = perm[slots]
            v = nodes >= 0
            dv = streams[k]["dinvT"].T.reshape(-1)
            blk = np.zeros((nrow, 128), np.float32)
            blk[v] = x[nodes[v]] * dv[slots[v], None]
            xs_all[col:col + nrow] = blk
            col += nrow
    xsb = xs_all.T.astype(bf16).copy()
    in_maps = []
    for k in range(N_CORES):
        s = streams[k]
        perm = s["perm"]
        xk = np.zeros((SHARD_PAD, 128), np.float32)
        v = perm >= 0
        xk[v] = x[perm[v]]
        in_maps.append(dict(
            xT=xk.T.astype(bf16).copy(),
            xsb=xsb,
            idx=s["idx"], idx2=s["idx2"], colr=np.asarray(s["colr"]),
            colrA=np.asarray(s["colrA"]), colrB=np.asarray(s["colrB"]),
            dinvT=s["dinvT"],
            iota=np.asarray(iota), ident=ident,
            w1i=w1i.astype(bf16), w1r=w1r.astype(bf16),
            w2i=w2i.astype(bf16), w2r=w2r.astype(bf16),
            b1b=b1b, b2b=b2b,
        ))
    return in_maps


def kernel(x, edge_index, w1_init, w1_root, b1, w2_init, w2_root, b2, **kw):
    global _LAST_RESULTS
    from concourse.bass_utils import run_bass_kernel_spmd

    inputs = dict(x=x, edge_index=edge_index, w1_init=w1_init, w1_root=w1_root,
                  b1=b1, w2_init=w2_init, w2_root=w2_root, b2=b2)
    streams, groups, TOT = _prep(np.asarray(edge_index))

    key = (groups, TOT)
    if key not in _PROG_CACHE:
        _PROG_CACHE[key] = _build_program(groups, TOT)
    nc = _PROG_CACHE[key]

    in_maps = build_in_maps(inputs, streams)

    import os
    trace = os.environ.get("BIARMA_TRACE", "0") == "1"
    res = run_bass_kernel_spmd(nc, in_maps, core_ids=list(range(N_CORES)),
                               trace=trace)
    _LAST_RESULTS = res

    out = np.zeros((N_NODES, CLS), np.float32)
    for k in range(N_CORES):
        o = res.results[k]["out"]
        perm = streams[k]["perm"]
        v = perm >= 0
        out[perm[v]] = o[v][:, :CLS]
    return out



# revision 11
# speedup vs baseline: 2.0620x; 1.0090x over previous
"""BiARMA (2-layer ARMAConv GNN) Trainium2 kernel, 8-core SPMD — v2.

Changes vs v1:
  * Chunked AllGathers (4 slot-chunks) overlapped with edge-phase compute:
    block b of the gather reads AG chunk b, so C-phase block 0 starts as soon
    as the first chunk lands. Blocks are slot-chunks of all 8 cores
    (core-major within chunk) instead of core pairs.
  * Edge phase is block-outer / group-inner with f32 accumulation in SBUF
    (PSUM per (group, block), DVE add into acc).
  * x arrives host-pre-transposed [128, SHARD_PAD] and stays SBUF-resident;
    Phase A needs no PE transposes / DVE copies.
  * h tables are written only in cols 0:64 (the matmul never reads 64:128).
  * AllGather outputs are addr_space="Shared" (fast HBM-HBM path).
  * Tokens NOT sorted by source (random order measured faster on HW).
"""
import numpy as np
import ml_dtypes

N_CORES = 8
N_NODES = 100000
IN_CH, HID, CLS = 128, 64, 40
SHARD = 12500
SHARD_PAD = 12544          # 98 * 128
NT = SHARD_PAD // 128      # dest tiles per core (98)
VPAD = N_CORES * SHARD_PAD

# slot-chunks (AllGather chunks == gather blocks), in tiles
CHUNK_TILES = [25, 25, 25, 23]
NBLK = len(CHUNK_TILES)
CHUNK_ROWS = [t * 128 for t in CHUNK_TILES]            # per-core rows per chunk
CHUNK_START = np.concatenate([[0], np.cumsum(CHUNK_ROWS)])[:-1]
BLK_ROWS = [N_CORES * r for r in CHUNK_ROWS]           # table rows per block
BLK_START = np.concatenate([[0], np.cumsum(BLK_ROWS)])[:-1]
BLK_TILE_START = np.concatenate(
    [[0], np.cumsum([CHUNK_TILES[c] * N_CORES for c in range(NBLK)])])[:-1]
assert max(BLK_ROWS) <= 32767

# group split per chunk: groups are position-contiguous tile runs; psC needs
# G*64 f32 <= 2 PSUM banks -> G <= 16. Light/heavy caps experiment showed no
# viable c=2 tiles at this density, so caps are uniform.
LIGHT_PER_CHUNK = [0, 0, 0, 0]
GROUP_SPLIT = []          # list of (ntiles, cap_idx)
for _c in range(NBLK):
    L = LIGHT_PER_CHUNK[_c]
    H = CHUNK_TILES[_c] - L
    if L:
        GROUP_SPLIT.append((L, 2))
    # split heavy run into chunks of <=13 tiles (psC <= 832 f32 cols)
    while H > 0:
        take = min(13, H)
        GROUP_SPLIT.append((take, 3))
        H -= take
assert sum(g for g, _ in GROUP_SPLIT) == NT
NQUEUES = 4

bf16 = ml_dtypes.bfloat16
SUBCALL = 1024

_PROG_CACHE = {}


# ----------------------------------------------------------------------------
# host-side prep
# ----------------------------------------------------------------------------

def _pack_tiles(cnt, ntiles, dests, caps=None):
    """Greedy balanced packing of `dests` into `ntiles` tiles of 128 slots.

    cnt: [len(dests), K] per-dest edge counts by source block (K>=1).
    caps: optional [ntiles] per-block token capacity per tile; the greedy
    minimizes max-over-K of (sums+cnt)/cap (relative fill).
    Returns tile_of, slot_of (len(dests)).
    """
    tot = cnt.sum(1)
    order = np.argsort(-tot, kind="stable")
    K = cnt.shape[1]
    sums = np.zeros((ntiles, K), np.float64)
    nd = np.zeros(ntiles, np.int64)
    tile_of = np.empty(len(dests), np.int32)
    slot_of = np.empty(len(dests), np.int32)
    capv = np.ones(ntiles) if caps is None else np.asarray(caps, np.float64)
    BIG = 1 << 40
    for i in order:
        load = ((sums + cnt[i]) / capv[:, None]).max(axis=1) + (nd >= 128) * BIG
        t = int(np.argmin(load))
        tile_of[i] = t
        slot_of[i] = nd[t]
        nd[t] += 1
        sums[t] += cnt[i]
    return tile_of, slot_of


def _prep(edge_index):
    """Host prep. Returns per-core stream dict + c_tb."""
    row = np.asarray(edge_index[0]).astype(np.int64)
    col = np.asarray(edge_index[1]).astype(np.int64)
    deg = np.bincount(col, minlength=N_NODES).astype(np.float64)
    dinv = np.where(deg > 0, 1.0 / np.sqrt(np.maximum(deg, 1e-12)), 0.0).astype(np.float32)

    src_core = row // SHARD
    dst_core = col // SHARD
    chunk_of_slot = np.searchsorted(CHUNK_START, np.arange(SHARD_PAD),
                                    side="right") - 1

    # ---- pass 1: pack dests by TOTAL degree -> fixes each node's slot-CHUNK.
    # (An edge's gather block = its SOURCE node's slot-chunk; pass 2 only
    # moves dests between tiles of the same chunk, so blocks stay fixed.)
    deg_dst = np.bincount(col, minlength=N_NODES)
    chunk1 = []   # per core: chunk of each local dest [SHARD_PAD]
    for k in range(N_CORES):
        cnt = np.zeros((SHARD_PAD, 1), np.int64)
        cnt[:SHARD, 0] = deg_dst[k * SHARD:(k + 1) * SHARD]
        tile_of, _ = _pack_tiles(cnt, NT, np.arange(SHARD_PAD))
        chunk1.append(chunk_of_slot[tile_of * 128])

    # edge -> source chunk (fixed after pass 1)
    src_chunk = np.empty(len(row), np.int64)
    for k in range(N_CORES):
        m = src_core == k
        src_chunk[m] = chunk1[k][row[m] - k * SHARD]

    # ---- pass 2: within each chunk, re-pack dests by per-block counts,
    # with light tiles (first LIGHT_PER_CHUNK positions) on a smaller cap.
    packs = []
    for k in range(N_CORES):
        m = dst_core == k
        ec = col[m] - k * SHARD
        eb = src_chunk[m]
        cnt = np.zeros((SHARD_PAD, NBLK), np.int64)
        np.add.at(cnt, (ec, eb), 1)
        tile_of = np.empty(SHARD_PAD, np.int32)
        slot_of = np.empty(SHARD_PAD, np.int32)
        t0 = 0
        for c in range(NBLK):
            dests = np.nonzero(chunk1[k] == c)[0]
            assert len(dests) == CHUNK_TILES[c] * 128
            L = LIGHT_PER_CHUNK[c]
            caps = np.array([256.0] * L + [512.0] * (CHUNK_TILES[c] - L))
            tf, sf = _pack_tiles(cnt[dests], CHUNK_TILES[c], dests, caps=caps)
            tile_of[dests] = t0 + tf
            slot_of[dests] = sf
            t0 += CHUNK_TILES[c]
        packs.append((tile_of, slot_of))

    # global slot of each node (pass-2 slots; chunks unchanged from pass 1)
    slot_global = np.empty(N_NODES, np.int64)
    perms = []
    for k in range(N_CORES):
        tile_of, slot_of = packs[k]
        slot_idx = tile_of * 128 + slot_of
        slot_global[k * SHARD:(k + 1) * SHARD] = slot_idx[:SHARD]
        perm = np.full(SHARD_PAD, -1, np.int64)
        d_local = np.arange(SHARD_PAD)
        node = k * SHARD + d_local
        valid = d_local < SHARD
        perm[slot_idx[valid]] = node[valid]
        perms.append(perm)

    src_slot = slot_global[row]
    # table row within block: core-major
    src_row_in_blk = src_core * np.asarray(CHUNK_ROWS)[src_chunk] + (
        src_slot - CHUNK_START[src_chunk])
    assert (src_chunk == chunk_of_slot[src_slot]).all()

    # ---- pass 2: per-core streams
    cores = []
    for k in range(N_CORES):
        m = dst_core == k
        er_row_in_blk = src_row_in_blk[m]
        eb = src_chunk[m]
        ec = col[m] - k * SHARD
        tile_of, slot_of = packs[k]
        et = tile_of[ec]
        eslot = slot_of[ec]
        cnt = np.zeros((NT, NBLK), np.int64)
        np.add.at(cnt, (et, eb), 1)
        cores.append((er_row_in_blk, eb, et, eslot, cnt))

    # resolve per-(group, block) run length from actual max counts across cores
    maxcnt = np.zeros((NT, NBLK), np.int64)
    for c in cores:
        maxcnt = np.maximum(maxcnt, c[4])
    gsizes = [g for g, _ in GROUP_SPLIT]
    g_starts = np.concatenate([[0], np.cumsum(gsizes)])[:-1]
    groups = []            # resolved (ntiles, (cg per block))
    for gi, (G, _cap) in enumerate(GROUP_SPLIT):
        t0 = int(g_starts[gi])
        cgs = tuple(
            max(1, int(np.ceil(maxcnt[t0:t0 + G, b].max() / 128)))
            for b in range(NBLK))
        groups.append((G, cgs))
    groups = tuple(groups)
    TOT = sum(G * cg * 128 for G, cgs in groups for cg in cgs)

    # token stream layout: [g][b][tiles of g][cg_gb*128]
    base_bt = np.empty((NBLK, NT), np.int64)
    run_bt = np.empty((NBLK, NT), np.int64)
    off = 0
    for gi, (G, cgs) in enumerate(groups):
        for b in range(NBLK):
            run = cgs[b] * 128
            for tp in range(G):
                t = int(g_starts[gi]) + tp
                base_bt[b, t] = off
                run_bt[b, t] = run
                off += run
    assert off == TOT

    streams = []
    for k in range(N_CORES):
        er_blkrow, eb, et, eslot, _cnt = cores[k]
        key = eb * NT + et
        order = np.argsort(key, kind="stable")
        ks = key[order]
        grp_start = np.searchsorted(ks, ks)
        rank = np.arange(len(ks)) - grp_start
        pos = base_bt[eb[order], et[order]] + rank
        tok_src = np.zeros(TOT, np.int16)
        tok_colr = np.full(TOT, 200.0, np.float32)
        tok_src[pos] = er_blkrow[order].astype(np.int16)
        tok_colr[pos] = eslot[order].astype(np.float32)
        # layer-2 packed stream: same positions, packed row + half-selected colr
        r = er_blkrow[order]
        packed = (r // 128) * 64 + (r % 64)
        half = (r % 128) // 64
        tok_src2 = np.zeros(TOT, np.int16)
        tok_src2[pos] = packed.astype(np.int16)
        tok_colrA = np.full(TOT, 200.0, np.float32)
        tok_colrB = np.full(TOT, 200.0, np.float32)
        es = eslot[order].astype(np.float32)
        pA_ = pos[half == 0]
        pB_ = pos[half == 1]
        tok_colrA[pA_] = es[half == 0]
        tok_colrB[pB_] = es[half == 1]
        # idx16 wrap: [16, TOT/16], transposed per 16-token groups
        idx16 = tok_src.reshape(TOT // 16, 16).T.copy()
        idx128 = np.tile(idx16, (8, 1)).copy()
        idx16b = tok_src2.reshape(TOT // 16, 16).T.copy()
        idx128b = np.tile(idx16b, (8, 1)).copy()
        colr = tok_colr.reshape(TOT // 128, 128).T.astype(bf16).copy()
        colrA = tok_colrA.reshape(TOT // 128, 128).T.astype(bf16).copy()
        colrB = tok_colrB.reshape(TOT // 128, 128).T.astype(bf16).copy()
        # dinv in slot order [128, NT]
        perm = perms[k]
        dinv_slot = np.zeros(SHARD_PAD, np.float32)
        v = perm >= 0
        dinv_slot[v] = dinv[perm[v]]
        dinvT = dinv_slot.reshape(NT, 128).T.copy()
        streams.append(dict(idx=idx128, idx2=idx128b, colr=colr,
                            colrA=colrA, colrB=colrB, dinvT=dinvT, perm=perm))
    return streams, groups, TOT


# ----------------------------------------------------------------------------
# device program
# ----------------------------------------------------------------------------

def _build_program(groups, TOT):
    import concourse.bacc as bacc
    import concourse.mybir as mybir
    import concourse.tile as tile
    from concourse import library_config

    f32 = mybir.dt.float32
    b16 = mybir.dt.bfloat16
    i16 = mybir.dt.int16
    Copy = mybir.ActivationFunctionType.Copy
    Relu = mybir.ActivationFunctionType.Relu
    ADD = mybir.AluOpType.add
    MULT = mybir.AluOpType.mult
    ISEQ = mybir.AluOpType.is_equal

    nc = bacc.Bacc("TRN2", target_bir_lowering=False, debug=False,
                   num_devices=N_CORES, num_swdge_queues=NQUEUES)

    t_xT = nc.dram_tensor("xT", [128, SHARD_PAD], b16, kind="ExternalInput")
    t_xsb = nc.dram_tensor("xsb", [128, VPAD], b16, kind="ExternalInput")
    t_idx = nc.dram_tensor("idx", [128, TOT // 16], i16, kind="ExternalInput")
    t_idx2 = nc.dram_tensor("idx2", [128, TOT // 16], i16, kind="ExternalInput")
    t_colr = nc.dram_tensor("colr", [128, TOT // 128], b16, kind="ExternalInput")
    t_colrA = nc.dram_tensor("colrA", [128, TOT // 128], b16, kind="ExternalInput")
    t_colrB = nc.dram_tensor("colrB", [128, TOT // 128], b16, kind="ExternalInput")
    t_dinv = nc.dram_tensor("dinvT", [128, NT], f32, kind="ExternalInput")
    t_iota = nc.dram_tensor("iota", [128, 128], b16, kind="ExternalInput")
    t_ident = nc.dram_tensor("ident", [128, 128], f32, kind="ExternalInput")
    t_w1i = nc.dram_tensor("w1i", [128, 64], b16, kind="ExternalInput")
    t_w1r = nc.dram_tensor("w1r", [128, 64], b16, kind="ExternalInput")
    t_w2i = nc.dram_tensor("w2i", [64, 64], b16, kind="ExternalInput")
    t_w2r = nc.dram_tensor("w2r", [64, 64], b16, kind="ExternalInput")
    t_b1 = nc.dram_tensor("b1b", [128, 64], f32, kind="ExternalInput")
    t_b2 = nc.dram_tensor("b2b", [128, 64], f32, kind="ExternalInput")
    t_out = nc.dram_tensor("out", [SHARD_PAD, 64], f32, kind="ExternalOutput")

    CHUNK_W = max(G * cg * 128 for G, cg in groups)   # max tokens per (g,b)
    g_starts = np.concatenate([[0], np.cumsum([g for g, _ in groups])])[:-1]

    with tile.TileContext(nc) as tc:
        with (
            tc.tile_pool(name="cst", bufs=1) as cst,
            tc.tile_pool(name="acc", bufs=1) as accp,
            tc.tile_pool(name="hb", bufs=4) as hbp,
            tc.tile_pool(name="rootA", bufs=NT) as rootA,
            tc.tile_pool(name="rootB", bufs=NT) as rootB,
            tc.tile_pool(name="sp", bufs=6) as sp,
            tc.tile_pool(name="sxp", bufs=3) as sxp,
            tc.tile_pool(name="mp", bufs=4) as mp,
            tc.tile_pool(name="ohp", bufs=3) as ohp,
            tc.tile_pool(name="h1p", bufs=4) as h1p,
            tc.tile_pool(name="op", bufs=4) as op_,
            tc.tile_pool(name="psC", bufs=2, space="PSUM") as psC,
            tc.tile_pool(name="psA", bufs=2, space="PSUM") as psA,
            tc.tile_pool(name="psR", bufs=1, space="PSUM") as psR,
            tc.tile_pool(name="dram", bufs=1, space="DRAM") as dram,
        ):
            nc.gpsimd.load_library(library_config.mlp)

            def load_const(t, shape, dt, tag):
                s = cst.tile(shape, dt, tag=tag, name=tag)
                nc.sync.dma_start(s[:], t[:])
                return s

            iota_s = load_const(t_iota, [128, 128], b16, tag="iota_s")
            ident_s = load_const(t_ident, [128, 128], f32, tag="ident_s")
            w1i_s = load_const(t_w1i, [128, 64], b16, tag="w1i_s")
            w1r_s = load_const(t_w1r, [128, 64], b16, tag="w1r_s")
            w2i_s = load_const(t_w2i, [64, 64], b16, tag="w2i_s")
            w2r_s = load_const(t_w2r, [64, 64], b16, tag="w2r_s")
            b1_s = load_const(t_b1, [128, 64], f32, tag="b1_s")
            b2_s = load_const(t_b2, [128, 64], f32, tag="b2_s")
            dinv_s = load_const(t_dinv, [128, NT], f32, tag="dinv_s")
            xT_s = load_const(t_xT, [128, SHARD_PAD], b16, tag="xT_s")
            idx_s = load_const(t_idx, [128, TOT // 16], i16, tag="idx_s")
            idx2_s = load_const(t_idx2, [128, TOT // 16], i16, tag="idx2_s")
            colr_s = load_const(t_colr, [128, TOT // 128], b16, tag="colr_s")
            colrA_s = load_const(t_colrA, [128, TOT // 128], b16, tag="colrA_s")
            colrB_s = load_const(t_colrB, [128, TOT // 128], b16, tag="colrB_s")

            hh_own = dram.tile([SHARD_PAD // 2, 128], b16)
            h_full = [dram.tile([BLK_ROWS[c], 128], b16,
                                name=f"h_full_{c}") for c in range(NBLK)]
            hh_full = [dram.tile([BLK_ROWS[c] // 2, 128], b16,
                                 addr_space="Shared",
                                 name=f"hh_full_{c}") for c in range(NBLK)]

            # ------- Phase A: replicated full-table build (no AG1) -------
            # xsb is dinv-scaled full x, columns pre-permuted into table-row
            # order (block-major, core-major). Stream 16 tiles per DMA.
            STREAM = 16
            GW = 4      # tiles per grouped table write (block sizes % 4 == 0)
            NT_ALL = VPAD // 128
            assert all((CHUNK_TILES[c] * N_CORES) % GW == 0 for c in range(NBLK))
            pos = 0
            while pos < NT_ALL:
                take = min(STREAM, NT_ALL - pos)
                sx = sxp.tile([128, STREAM * 128], b16, tag="sx")
                nc.sync.dma_start(sx[:, 0:take * 128],
                                  t_xsb[:, pos * 128:(pos + take) * 128])
                for j0 in range(0, take, GW):
                    wide = hbp.tile([128, GW * 64], b16, tag="hbw")
                    pA4 = psA.tile([128, GW * 64], f32, tag="psA4")
                    for j in range(j0, j0 + GW):
                        nc.tensor.matmul(out=pA4[:, (j - j0) * 64:
                                                  (j - j0 + 1) * 64],
                                         lhsT=sx[:, j * 128:(j + 1) * 128],
                                         rhs=w1i_s[:], start=True, stop=True)
                    nc.scalar.activation(wide[:, :], pA4[:, :], Copy)
                    t0g = pos + j0
                    c = int(np.searchsorted(BLK_TILE_START, t0g,
                                            side="right")) - 1
                    rr = (t0g - int(BLK_TILE_START[c])) * 128
                    nc.sync.dma_start(
                        h_full[c][rr:rr + GW * 128, 0:64].rearrange(
                            "(j p) f -> p j f", j=GW),
                        wide[:, :].rearrange("p (j f) -> p j f", j=GW))
                pos += take

            # ------- own-shard root1 (x @ w1_root + b1) -------
            root1 = []
            for t in range(NT):
                lhsT = xT_s[:, t * 128:(t + 1) * 128]
                pA = psA.tile([128, GW * 64], f32, tag="psA4")
                nc.tensor.matmul(out=pA[:, 64:128], lhsT=lhsT, rhs=w1r_s[:],
                                 start=True, stop=True)
                r1 = rootA.tile([128, 64], b16, tag="rootA")
                nc.vector.tensor_tensor(out=r1[:], in0=pA[:, 64:128],
                                        in1=b1_s[:], op=ADD)
                root1.append(r1)

            # -------- edge phase: group-outer, block-inner PSUM chains --------
            gsizes = [G for G, _ in groups]
            g_starts = np.concatenate([[0], np.cumsum(gsizes)])[:-1]
            CHUNK_W = max(G * cg * 128 for G, cgs in groups for cg in cgs)
            # absolute token offsets per (g, b), mirroring _prep layout
            off_gb = {}
            _off = 0
            for gi, (G, cgs) in enumerate(groups):
                for b in range(NBLK):
                    off_gb[(gi, b)] = _off
                    _off += G * cgs[b] * 128

            qstate = [0]

            def edge_group(table, gi, packed=False):
                G, cgs = groups[gi]
                acc = sp.tile([128, G * 64], f32, tag="eacc")
                my_idx = idx2_s if packed else idx_s
                for b in range(NBLK):
                    pC = psC.tile([128, G * 64], f32, tag="psC")
                    n = G * cgs[b] * 128
                    nch = G * cgs[b]
                    off = off_gb[(gi, b)]
                    jg = off // 128
                    m = mp.tile([128, CHUNK_W], b16, tag="mp")
                    done = 0
                    while done < n:
                        step = min(SUBCALL, n - done)
                        nc.gpsimd.dma_gather(
                            out_ap=m[:, done:done + step].rearrange(
                                "p (c d) -> p c d", d=128),
                            in_ap=table[b][:, :],
                            idxs_ap=my_idx[:, (off + done) // 16:
                                           (off + done + step) // 16],
                            num_idxs=step,
                            num_idxs_reg=step,
                            elem_size=128,
                            queue_num=qstate[0] % NQUEUES,
                        )
                        qstate[0] += 1
                        done += step
                    if not packed:
                        oh = ohp.tile([128, CHUNK_W], b16, tag="ohp")
                        nc.vector.tensor_tensor(
                            out=oh[:, :n].rearrange("p (c d) -> p c d", d=128),
                            in0=iota_s[:].unsqueeze(1).broadcast_to(
                                [128, nch, 128]),
                            in1=colr_s[:, jg:jg + nch].unsqueeze(2).broadcast_to(
                                [128, nch, 128]),
                            op=ISEQ,
                        )
                        for ti in range(G):
                            for cc in range(cgs[b]):
                                jj = (ti * cgs[b] + cc) * 128
                                nc.tensor.matmul(
                                    out=pC[:, ti * 64:(ti + 1) * 64],
                                    lhsT=oh[:, jj:jj + 128],
                                    rhs=m[:, jj:jj + 64],
                                    start=(cc == 0),
                                    stop=(cc == cgs[b] - 1),
                                )
                    else:
                        ohA = ohp.tile([128, CHUNK_W], b16, tag="ohp")
                        nc.vector.tensor_tensor(
                            out=ohA[:, :n].rearrange("p (c d) -> p c d", d=128),
                            in0=iota_s[:].unsqueeze(1).broadcast_to(
                                [128, nch, 128]),
                            in1=colrA_s[:, jg:jg + nch].unsqueeze(2).broadcast_to(
                                [128, nch, 128]),
                            op=ISEQ,
                        )
                        ohB = ohp.tile([128, CHUNK_W], b16, tag="ohp")
                        nc.vector.tensor_tensor(
                            out=ohB[:, :n].rearrange("p (c d) -> p c d", d=128),
                            in0=iota_s[:].unsqueeze(1).broadcast_to(
                                [128, nch, 128]),
                            in1=colrB_s[:, jg:jg + nch].unsqueeze(2).broadcast_to(
                                [128, nch, 128]),
                            op=ISEQ,
                        )
                        for ti in range(G):
                            for cc in range(cgs[b]):
                                jj = (ti * cgs[b] + cc) * 128
                                nc.tensor.matmul(
                                    out=pC[:, ti * 64:(ti + 1) * 64],
                                    lhsT=ohA[:, jj:jj + 128],
                                    rhs=m[:, jj:jj + 64],
                                    start=(cc == 0),
                                    stop=False,
                                )
                                nc.tensor.matmul(
                                    out=pC[:, ti * 64:(ti + 1) * 64],
                                    lhsT=ohB[:, jj:jj + 128],
                                    rhs=m[:, jj + 64:jj + 128],
                                    start=False,
                                    stop=(cc == cgs[b] - 1),
                                )
                    if b == 0:
                        nc.vector.tensor_copy(out=acc[:], in_=pC[:])
                    else:
                        nc.vector.tensor_tensor(out=acc[:], in0=acc[:],
                                                in1=pC[:], op=ADD)
                return acc

            groups_of_chunk = []
            _gi = 0
            for c in range(NBLK):
                tiles_left = CHUNK_TILES[c]
                lst = []
                while tiles_left > 0:
                    lst.append(_gi)
                    tiles_left -= groups[_gi][0]
                    _gi += 1
                groups_of_chunk.append(lst)

            # ---------------- C1 + D + chunked AG2 ----------------
            root2 = []
            for c in range(NBLK):
                for gi in groups_of_chunk[c]:
                    G = groups[gi][0]
                    pC = edge_group(h_full, gi)
                    for tp in range(G):
                        t = int(g_starts[gi]) + tp
                        s1 = sp.tile([128, 64], f32, tag="s1")
                        nc.scalar.activation(s1[:], pC[:, tp * 64:(tp + 1) * 64],
                                             Copy, scale=dinv_s[:, t:t + 1])
                        s2 = sp.tile([128, 64], f32, tag="s2")
                        nc.vector.tensor_tensor(out=s2[:], in0=s1[:],
                                                in1=root1[t][:], op=ADD)
                        pT = psR.tile([128, 128], f32, tag="psT")
                        nc.tensor.transpose(out=pT[:64, :], in_=s2[:],
                                            identity=ident_s[:])
                        h1t = h1p.tile([64, 128], b16, tag="h1t")
                        nc.scalar.activation(h1t[:], pT[:64, :], Relu)
                        pDB = psR.tile([128, 128], f32, tag="psDB")
                        nc.tensor.matmul(out=pDB[:, 0:64], lhsT=h1t[:],
                                         rhs=w2i_s[:], start=True, stop=True)
                        nc.tensor.matmul(out=pDB[:, 64:128], lhsT=h1t[:],
                                         rhs=w2r_s[:], start=True, stop=True)
                        hht = hbp.tile([128, 64], b16, tag="hb2")
                        nc.scalar.activation(hht[:], pDB[:, 0:64], Copy,
                                             scale=dinv_s[:, t:t + 1])
                        nc.sync.dma_start(
                            hh_own[t * 64:(t + 1) * 64, 0:64], hht[0:64, :])
                        nc.sync.dma_start(
                            hh_own[t * 64:(t + 1) * 64, 64:128], hht[64:128, :])
                        r2 = rootB.tile([128, 64], b16, tag="rootB")
                        nc.vector.tensor_tensor(out=r2[:], in0=pDB[:, 64:128],
                                                in1=b2_s[:], op=ADD)
                        root2.append(r2)
                r0 = int(CHUNK_START[c]) // 2
                r1_ = r0 + CHUNK_ROWS[c] // 2
                nc.gpsimd.collective_compute(
                    "AllGather", mybir.AluOpType.bypass,
                    replica_groups=[list(range(N_CORES))],
                    ins=[hh_own[r0:r1_, :].opt()], outs=[hh_full[c][:].opt()],
                )

            # ---------------- C2 -> output ----------------
            for gi in range(len(groups)):
                G = groups[gi][0]
                pC = edge_group(hh_full, gi, packed=True)
                for tp in range(G):
                    t = int(g_starts[gi]) + tp
                    s1 = sp.tile([128, 64], f32, tag="s1b")
                    nc.scalar.activation(s1[:], pC[:, tp * 64:(tp + 1) * 64],
                                         Copy, scale=dinv_s[:, t:t + 1])
                    s2 = sp.tile([128, 64], f32, tag="s2b")
                    nc.vector.tensor_tensor(out=s2[:], in0=s1[:],
                                            in1=root2[t][:], op=ADD)
                    o = op_.tile([128, 64], f32, tag="o")
                    nc.scalar.activation(o[:], s2[:], Relu)
                    nc.sync.dma_start(t_out[t * 128:(t + 1) * 128, :], o[:])

    nc.compile()
    return nc


# ----------------------------------------------------------------------------
# entry point
# ----------------------------------------------------------------------------

_LAST_RESULTS = None


def build_in_maps(inputs, streams):
    x = np.asarray(inputs["x"], np.float32)
    iota = np.broadcast_to(np.arange(128, dtype=np.float32), (128, 128)).astype(bf16)
    ident = np.eye(128, dtype=np.float32)
    w1i = np.asarray(inputs["w1_init"], np.float32)
    w1r = np.asarray(inputs["w1_root"], np.float32)
    w2i = np.zeros((64, 64), np.float32); w2i[:, :CLS] = inputs["w2_init"]
    w2r = np.zeros((64, 64), np.float32); w2r[:, :CLS] = inputs["w2_root"]
    b1b = np.broadcast_to(np.asarray(inputs["b1"], np.float32), (128, HID)).copy()
    b2p = np.zeros(64, np.float32); b2p[:CLS] = inputs["b2"]
    b2b = np.broadcast_to(b2p, (128, 64)).copy()
    xs_all = np.zeros((VPAD, 128), np.float32)
    col = 0
    for c in range(NBLK):
        for k in range(N_CORES):
            s0 = int(CHUNK_START[c]); nrow = CHUNK_ROWS[c]
            slots = np.arange(s0, s0 + nrow)
            perm = streams[k]["perm"]
            nodes = perm[slots]
            v = nodes >= 0
            dv = streams[k]["dinvT"].T.reshape(-1)
            blk = np.zeros((nrow, 128), np.float32)
            blk[v] = x[nodes[v]] * dv[slots[v], None]
            xs_all[col:col + nrow] = blk
            col += nrow
    xsb = xs_all.T.astype(bf16).copy()
    in_maps = []
    for k in range(N_CORES):
        s = streams[k]
        perm = s["perm"]
        xk = np.zeros((SHARD_PAD, 128), np.float32)
        v = perm >= 0
        xk[v] = x[perm[v]]
        in_maps.append(dict(
            xT=xk.T.astype(bf16).copy(),
            xsb=xsb,
            idx=s["idx"], idx2=s["idx2"], colr=np.asarray(s["colr"]),
            colrA=np.asarray(s["colrA"]), colrB=np.asarray(s["colrB"]),
            dinvT=s["dinvT"],
            iota=np.asarray(iota), ident=ident,
            w1i=w1i.astype(bf16), w1r=w1r.astype(bf16),
            w2i=w2i.astype(bf16), w2r=w2r.astype(bf16),
            b1b=b1b, b2b=b2b,
        ))
    return in_maps


def kernel(x, edge_index, w1_init, w1_root, b1, w2_init, w2_root, b2, **kw):
    global _LAST_RESULTS
    from concourse.bass_utils import run_bass_kernel_spmd

    inputs = dict(x=x, edge_index=edge_index, w1_init=w1_init, w1_root=w1_root,
                  b1=b1, w2_init=w2_init, w2_root=w2_root, b2=b2)
    streams, groups, TOT = _prep(np.asarray(edge_index))

    key = (groups, TOT)
    if key not in _PROG_CACHE:
        _PROG_CACHE[key] = _build_program(groups, TOT)
    nc = _PROG_CACHE[key]

    in_maps = build_in_maps(inputs, streams)

    import os
    trace = os.environ.get("BIARMA_TRACE", "0") == "1"
    res = run_bass_kernel_spmd(nc, in_maps, core_ids=list(range(N_CORES)),
                               trace=trace)
    _LAST_RESULTS = res

    out = np.zeros((N_NODES, CLS), np.float32)
    for k in range(N_CORES):
        o = res.results[k]["out"]
        perm = streams[k]["perm"]
        v = perm >= 0
        out[perm[v]] = o[v][:, :CLS]
    return out



# revision 14
# speedup vs baseline: 2.2796x; 1.1055x over previous
"""BiARMA (2-layer ARMAConv GNN) Trainium2 kernel, 8-core SPMD — v2.

Changes vs v1:
  * Chunked AllGathers (4 slot-chunks) overlapped with edge-phase compute:
    block b of the gather reads AG chunk b, so C-phase block 0 starts as soon
    as the first chunk lands. Blocks are slot-chunks of all 8 cores
    (core-major within chunk) instead of core pairs.
  * Edge phase is block-outer / group-inner with f32 accumulation in SBUF
    (PSUM per (group, block), DVE add into acc).
  * x arrives host-pre-transposed [128, SHARD_PAD] and stays SBUF-resident;
    Phase A needs no PE transposes / DVE copies.
  * h tables are written only in cols 0:64 (the matmul never reads 64:128).
  * AllGather outputs are addr_space="Shared" (fast HBM-HBM path).
  * Tokens NOT sorted by source (random order measured faster on HW).
"""
import numpy as np
import ml_dtypes

N_CORES = 8
N_NODES = 100000
IN_CH, HID, CLS = 128, 64, 40
SHARD = 12500
SHARD_PAD = 12544          # 98 * 128
NT = SHARD_PAD // 128      # dest tiles per core (98)
VPAD = N_CORES * SHARD_PAD

# slot-chunks (AllGather chunks == gather blocks), in tiles
CHUNK_TILES = [25, 25, 25, 23]
NBLK = len(CHUNK_TILES)
CHUNK_ROWS = [t * 128 for t in CHUNK_TILES]            # per-core rows per chunk
CHUNK_START = np.concatenate([[0], np.cumsum(CHUNK_ROWS)])[:-1]
BLK_ROWS = [N_CORES * r for r in CHUNK_ROWS]           # table rows per block
BLK_START = np.concatenate([[0], np.cumsum(BLK_ROWS)])[:-1]
BLK_TILE_START = np.concatenate(
    [[0], np.cumsum([CHUNK_TILES[c] * N_CORES for c in range(NBLK)])])[:-1]
assert max(BLK_ROWS) <= 32767

# group split per chunk: groups are position-contiguous tile runs; psC needs
# G*64 f32 <= 2 PSUM banks -> G <= 16. Light/heavy caps experiment showed no
# viable c=2 tiles at this density, so caps are uniform.
LIGHT_PER_CHUNK = [0, 0, 0, 0]
GROUP_SPLIT = []          # list of (ntiles, cap_idx)
for _c in range(NBLK):
    L = LIGHT_PER_CHUNK[_c]
    H = CHUNK_TILES[_c] - L
    if L:
        GROUP_SPLIT.append((L, 2))
    # split heavy run into chunks of <=13 tiles (psC <= 832 f32 cols)
    while H > 0:
        take = min(13, H)
        GROUP_SPLIT.append((take, 3))
        H -= take
assert sum(g for g, _ in GROUP_SPLIT) == NT
NQUEUES = 4

bf16 = ml_dtypes.bfloat16
SUBCALL = 1024

_PROG_CACHE = {}


# ----------------------------------------------------------------------------
# host-side prep
# ----------------------------------------------------------------------------

def _pack_tiles(cnt, ntiles, dests, caps=None):
    """Greedy balanced packing of `dests` into `ntiles` tiles of 128 slots.

    cnt: [len(dests), K] per-dest edge counts by source block (K>=1).
    caps: optional [ntiles] per-block token capacity per tile; the greedy
    minimizes max-over-K of (sums+cnt)/cap (relative fill).
    Returns tile_of, slot_of (len(dests)).
    """
    tot = cnt.sum(1)
    order = np.argsort(-tot, kind="stable")
    K = cnt.shape[1]
    sums = np.zeros((ntiles, K), np.float64)
    nd = np.zeros(ntiles, np.int64)
    tile_of = np.empty(len(dests), np.int32)
    slot_of = np.empty(len(dests), np.int32)
    capv = np.ones(ntiles) if caps is None else np.asarray(caps, np.float64)
    BIG = 1 << 40
    for i in order:
        load = ((sums + cnt[i]) / capv[:, None]).max(axis=1) + (nd >= 128) * BIG
        t = int(np.argmin(load))
        tile_of[i] = t
        slot_of[i] = nd[t]
        nd[t] += 1
        sums[t] += cnt[i]
    return tile_of, slot_of


def _prep(edge_index):
    """Host prep. Returns per-core stream dict + c_tb."""
    row = np.asarray(edge_index[0]).astype(np.int64)
    col = np.asarray(edge_index[1]).astype(np.int64)
    deg = np.bincount(col, minlength=N_NODES).astype(np.float64)
    dinv = np.where(deg > 0, 1.0 / np.sqrt(np.maximum(deg, 1e-12)), 0.0).astype(np.float32)

    src_core = row // SHARD
    dst_core = col // SHARD
    chunk_of_slot = np.searchsorted(CHUNK_START, np.arange(SHARD_PAD),
                                    side="right") - 1

    # ---- pass 1: pack dests by TOTAL degree -> fixes each node's slot-CHUNK.
    # (An edge's gather block = its SOURCE node's slot-chunk; pass 2 only
    # moves dests between tiles of the same chunk, so blocks stay fixed.)
    deg_dst = np.bincount(col, minlength=N_NODES)
    chunk1 = []   # per core: chunk of each local dest [SHARD_PAD]
    for k in range(N_CORES):
        cnt = np.zeros((SHARD_PAD, 1), np.int64)
        cnt[:SHARD, 0] = deg_dst[k * SHARD:(k + 1) * SHARD]
        tile_of, _ = _pack_tiles(cnt, NT, np.arange(SHARD_PAD))
        chunk1.append(chunk_of_slot[tile_of * 128])

    # edge -> source chunk (fixed after pass 1)
    src_chunk = np.empty(len(row), np.int64)
    for k in range(N_CORES):
        m = src_core == k
        src_chunk[m] = chunk1[k][row[m] - k * SHARD]

    # ---- pass 2: within each chunk, re-pack dests by per-block counts,
    # with light tiles (first LIGHT_PER_CHUNK positions) on a smaller cap.
    packs = []
    for k in range(N_CORES):
        m = dst_core == k
        ec = col[m] - k * SHARD
        eb = src_chunk[m]
        cnt = np.zeros((SHARD_PAD, NBLK), np.int64)
        np.add.at(cnt, (ec, eb), 1)
        tile_of = np.empty(SHARD_PAD, np.int32)
        slot_of = np.empty(SHARD_PAD, np.int32)
        t0 = 0
        for c in range(NBLK):
            dests = np.nonzero(chunk1[k] == c)[0]
            assert len(dests) == CHUNK_TILES[c] * 128
            L = LIGHT_PER_CHUNK[c]
            caps = np.array([256.0] * L + [512.0] * (CHUNK_TILES[c] - L))
            tf, sf = _pack_tiles(cnt[dests], CHUNK_TILES[c], dests, caps=caps)
            tile_of[dests] = t0 + tf
            slot_of[dests] = sf
            t0 += CHUNK_TILES[c]
        packs.append((tile_of, slot_of))

    # global slot of each node (pass-2 slots; chunks unchanged from pass 1)
    slot_global = np.empty(N_NODES, np.int64)
    perms = []
    for k in range(N_CORES):
        tile_of, slot_of = packs[k]
        slot_idx = tile_of * 128 + slot_of
        slot_global[k * SHARD:(k + 1) * SHARD] = slot_idx[:SHARD]
        perm = np.full(SHARD_PAD, -1, np.int64)
        d_local = np.arange(SHARD_PAD)
        node = k * SHARD + d_local
        valid = d_local < SHARD
        perm[slot_idx[valid]] = node[valid]
        perms.append(perm)

    src_slot = slot_global[row]
    # table row within block: core-major
    src_row_in_blk = src_core * np.asarray(CHUNK_ROWS)[src_chunk] + (
        src_slot - CHUNK_START[src_chunk])
    assert (src_chunk == chunk_of_slot[src_slot]).all()

    # ---- pass 2: per-core streams
    cores = []
    for k in range(N_CORES):
        m = dst_core == k
        er_row_in_blk = src_row_in_blk[m]
        eb = src_chunk[m]
        ec = col[m] - k * SHARD
        tile_of, slot_of = packs[k]
        et = tile_of[ec]
        eslot = slot_of[ec]
        cnt = np.zeros((NT, NBLK), np.int64)
        np.add.at(cnt, (et, eb), 1)
        cores.append((er_row_in_blk, eb, et, eslot, cnt))

    # resolve per-(group, block) run length from actual max counts across cores
    maxcnt = np.zeros((NT, NBLK), np.int64)
    for c in cores:
        maxcnt = np.maximum(maxcnt, c[4])
    gsizes = [g for g, _ in GROUP_SPLIT]
    g_starts = np.concatenate([[0], np.cumsum(gsizes)])[:-1]
    groups = []            # resolved (ntiles, (cg per block))
    for gi, (G, _cap) in enumerate(GROUP_SPLIT):
        t0 = int(g_starts[gi])
        cgs = tuple(
            max(1, int(np.ceil(maxcnt[t0:t0 + G, b].max() / 128)))
            for b in range(NBLK))
        groups.append((G, cgs))
    groups = tuple(groups)
    TOT = sum(G * cg * 128 for G, cgs in groups for cg in cgs)

    # token stream layout: [g][b][tiles of g][cg_gb*128]
    base_bt = np.empty((NBLK, NT), np.int64)
    run_bt = np.empty((NBLK, NT), np.int64)
    off = 0
    for gi, (G, cgs) in enumerate(groups):
        for b in range(NBLK):
            run = cgs[b] * 128
            for tp in range(G):
                t = int(g_starts[gi]) + tp
                base_bt[b, t] = off
                run_bt[b, t] = run
                off += run
    assert off == TOT

    streams = []
    for k in range(N_CORES):
        er_blkrow, eb, et, eslot, _cnt = cores[k]
        key = eb * NT + et
        order = np.argsort(key, kind="stable")
        ks = key[order]
        grp_start = np.searchsorted(ks, ks)
        rank = np.arange(len(ks)) - grp_start
        pos = base_bt[eb[order], et[order]] + rank
        tok_src = np.zeros(TOT, np.int16)
        tok_colr = np.full(TOT, 200.0, np.float32)
        tok_src[pos] = er_blkrow[order].astype(np.int16)
        tok_colr[pos] = eslot[order].astype(np.float32)
        # layer-2 packed stream: same positions, packed row + half-selected colr
        r = er_blkrow[order]
        packed = (r // 128) * 64 + (r % 64)
        half = (r % 128) // 64
        tok_src2 = np.zeros(TOT, np.int16)
        tok_src2[pos] = packed.astype(np.int16)
        tok_colrA = np.full(TOT, 200.0, np.float32)
        tok_colrB = np.full(TOT, 200.0, np.float32)
        es = eslot[order].astype(np.float32)
        pA_ = pos[half == 0]
        pB_ = pos[half == 1]
        tok_colrA[pA_] = es[half == 0]
        tok_colrB[pB_] = es[half == 1]
        # idx16 wrap: [16, TOT/16], transposed per 16-token groups
        idx16 = tok_src.reshape(TOT // 16, 16).T.copy()
        idx128 = np.tile(idx16, (8, 1)).copy()
        idx16b = tok_src2.reshape(TOT // 16, 16).T.copy()
        idx128b = np.tile(idx16b, (8, 1)).copy()
        colr = tok_colr.reshape(TOT // 128, 128).T.astype(bf16).copy()
        colrA = tok_colrA.reshape(TOT // 128, 128).T.astype(bf16).copy()
        colrB = tok_colrB.reshape(TOT // 128, 128).T.astype(bf16).copy()
        # dinv in slot order [128, NT]
        perm = perms[k]
        dinv_slot = np.zeros(SHARD_PAD, np.float32)
        v = perm >= 0
        dinv_slot[v] = dinv[perm[v]]
        dinvT = dinv_slot.reshape(NT, 128).T.copy()
        streams.append(dict(idx=idx128, idx2=idx128b, colr=colr,
                            colrA=colrA, colrB=colrB, dinvT=dinvT, perm=perm))
    return streams, groups, TOT


# ----------------------------------------------------------------------------
# device program
# ----------------------------------------------------------------------------

def _build_program(groups, TOT):
    import concourse.bacc as bacc
    import concourse.mybir as mybir
    import concourse.tile as tile
    from concourse import library_config

    f32 = mybir.dt.float32
    b16 = mybir.dt.bfloat16
    i16 = mybir.dt.int16
    Copy = mybir.ActivationFunctionType.Copy
    Relu = mybir.ActivationFunctionType.Relu
    ADD = mybir.AluOpType.add
    MULT = mybir.AluOpType.mult
    ISEQ = mybir.AluOpType.is_equal

    nc = bacc.Bacc("TRN2", target_bir_lowering=False, debug=False,
                   num_devices=N_CORES, num_swdge_queues=NQUEUES)

    t_xT = nc.dram_tensor("xT", [128, SHARD_PAD], b16, kind="ExternalInput")
    t_xsb = nc.dram_tensor("xsb", [128, VPAD], b16, kind="ExternalInput")
    t_idx = nc.dram_tensor("idx", [128, TOT // 16], i16, kind="ExternalInput")
    t_idx2 = nc.dram_tensor("idx2", [128, TOT // 16], i16, kind="ExternalInput")
    t_colr = nc.dram_tensor("colr", [128, TOT // 128], b16, kind="ExternalInput")
    t_colrA = nc.dram_tensor("colrA", [128, TOT // 128], b16, kind="ExternalInput")
    t_colrB = nc.dram_tensor("colrB", [128, TOT // 128], b16, kind="ExternalInput")
    t_dinv = nc.dram_tensor("dinvT", [128, NT], f32, kind="ExternalInput")
    t_iota = nc.dram_tensor("iota", [128, 128], b16, kind="ExternalInput")
    t_ident = nc.dram_tensor("ident", [128, 128], f32, kind="ExternalInput")
    t_w1i = nc.dram_tensor("w1i", [128, 64], b16, kind="ExternalInput")
    t_w1r = nc.dram_tensor("w1r", [128, 64], b16, kind="ExternalInput")
    t_w2i = nc.dram_tensor("w2i", [64, 64], b16, kind="ExternalInput")
    t_w2r = nc.dram_tensor("w2r", [64, 64], b16, kind="ExternalInput")
    t_b1 = nc.dram_tensor("b1b", [128, 64], f32, kind="ExternalInput")
    t_b2 = nc.dram_tensor("b2b", [128, 64], f32, kind="ExternalInput")
    t_out = nc.dram_tensor("out", [SHARD_PAD, 64], f32, kind="ExternalOutput")

    CHUNK_W = max(G * cg * 128 for G, cg in groups)   # max tokens per (g,b)
    g_starts = np.concatenate([[0], np.cumsum([g for g, _ in groups])])[:-1]

    with tile.TileContext(nc) as tc:
        with (
            tc.tile_pool(name="cst", bufs=1) as cst,
            tc.tile_pool(name="acc", bufs=1) as accp,
            tc.tile_pool(name="hb", bufs=4) as hbp,
            tc.tile_pool(name="rootA", bufs=NT) as rootA,
            tc.tile_pool(name="rootB", bufs=NT) as rootB,
            tc.tile_pool(name="sp", bufs=6) as sp,
            tc.tile_pool(name="sxp", bufs=2) as sxp,
            tc.tile_pool(name="mp", bufs=5) as mp,
            tc.tile_pool(name="ohp", bufs=3) as ohp,
            tc.tile_pool(name="h1p", bufs=4) as h1p,
            tc.tile_pool(name="op", bufs=4) as op_,
            tc.tile_pool(name="psC", bufs=2, space="PSUM") as psC,
            tc.tile_pool(name="psA", bufs=2, space="PSUM") as psA,
            tc.tile_pool(name="psR", bufs=1, space="PSUM") as psR,
            tc.tile_pool(name="dram", bufs=1, space="DRAM") as dram,
        ):
            nc.gpsimd.load_library(library_config.mlp)

            def load_const(t, shape, dt, tag):
                s = cst.tile(shape, dt, tag=tag, name=tag)
                nc.sync.dma_start(s[:], t[:])
                return s

            iota_s = load_const(t_iota, [128, 128], b16, tag="iota_s")
            ident_s = load_const(t_ident, [128, 128], f32, tag="ident_s")
            w1i_s = load_const(t_w1i, [128, 64], b16, tag="w1i_s")
            w1r_s = load_const(t_w1r, [128, 64], b16, tag="w1r_s")
            w2i_s = load_const(t_w2i, [64, 64], b16, tag="w2i_s")
            w2r_s = load_const(t_w2r, [64, 64], b16, tag="w2r_s")
            b1_s = load_const(t_b1, [128, 64], f32, tag="b1_s")
            b2_s = load_const(t_b2, [128, 64], f32, tag="b2_s")
            dinv_s = load_const(t_dinv, [128, NT], f32, tag="dinv_s")
            xT_s = load_const(t_xT, [128, SHARD_PAD], b16, tag="xT_s")
            idx_s = load_const(t_idx, [128, TOT // 16], i16, tag="idx_s")
            idx2_s = load_const(t_idx2, [128, TOT // 16], i16, tag="idx2_s")
            colr_s = load_const(t_colr, [128, TOT // 128], b16, tag="colr_s")
            colrA_s = load_const(t_colrA, [128, TOT // 128], b16, tag="colrA_s")
            colrB_s = load_const(t_colrB, [128, TOT // 128], b16, tag="colrB_s")

            hh_own = dram.tile([SHARD_PAD // 2, 128], b16)
            h_full = [dram.tile([BLK_ROWS[c], 128], b16,
                                name=f"h_full_{c}") for c in range(NBLK)]
            hh_full = [dram.tile([BLK_ROWS[c] // 2, 128], b16,
                                 addr_space="Shared",
                                 name=f"hh_full_{c}") for c in range(NBLK)]

            STREAM = 32
            GW = 8      # tiles per grouped table write
            NT_ALL = VPAD // 128
            assert all((CHUNK_TILES[c] * N_CORES) % GW == 0 for c in range(NBLK))

            # ------- own-shard root1 (x @ w1_root + b1) -------
            root1 = []
            for t in range(NT):
                lhsT = xT_s[:, t * 128:(t + 1) * 128]
                pA = psA.tile([128, GW * 64], f32, tag="psA4")
                nc.tensor.matmul(out=pA[:, 64:128], lhsT=lhsT, rhs=w1r_s[:],
                                 start=True, stop=True)
                r1 = rootA.tile([128, 64], b16, tag="rootA")
                nc.vector.tensor_tensor(out=r1[:], in0=pA[:, 64:128],
                                        in1=b1_s[:], op=ADD)
                root1.append(r1)

            # ------- Phase A: replicated full-table build (no AG1) -------
            # xsb is dinv-scaled full x, columns pre-permuted into table-row
            # order (block-major, core-major).
            pos = 0
            while pos < NT_ALL:
                take = min(STREAM, NT_ALL - pos)
                sx = sxp.tile([128, STREAM * 128], b16, tag="sx")
                nc.sync.dma_start(sx[:, 0:take * 128],
                                  t_xsb[:, pos * 128:(pos + take) * 128])
                for j0 in range(0, take, GW):
                    wide = hbp.tile([128, GW * 64], b16, tag="hbw")
                    pA4 = psA.tile([128, GW * 64], f32, tag="psA4")
                    for j in range(j0, j0 + GW):
                        nc.tensor.matmul(out=pA4[:, (j - j0) * 64:
                                                  (j - j0 + 1) * 64],
                                         lhsT=sx[:, j * 128:(j + 1) * 128],
                                         rhs=w1i_s[:], start=True, stop=True)
                    nc.scalar.activation(wide[:, :], pA4[:, :], Copy)
                    t0g = pos + j0
                    c = int(np.searchsorted(BLK_TILE_START, t0g,
                                            side="right")) - 1
                    rr = (t0g - int(BLK_TILE_START[c])) * 128
                    nc.sync.dma_start(
                        h_full[c][rr:rr + GW * 128, 0:64].rearrange(
                            "(j p) f -> p j f", j=GW),
                        wide[:, :].rearrange("p (j f) -> p j f", j=GW))
                pos += take


            # -------- edge phase: group-outer, block-inner PSUM chains --------
            gsizes = [G for G, _ in groups]
            g_starts = np.concatenate([[0], np.cumsum(gsizes)])[:-1]
            CHUNK_W = max(G * cg * 128 for G, cgs in groups for cg in cgs)
            # absolute token offsets per (g, b), mirroring _prep layout
            off_gb = {}
            _off = 0
            for gi, (G, cgs) in enumerate(groups):
                for b in range(NBLK):
                    off_gb[(gi, b)] = _off
                    _off += G * cgs[b] * 128

            qstate = [0]

            def edge_group(table, gi, packed=False):
                G, cgs = groups[gi]
                acc = sp.tile([128, G * 64], f32, tag="eacc")
                my_idx = idx2_s if packed else idx_s
                for b in range(NBLK):
                    pC = psC.tile([128, G * 64], f32, tag="psC")
                    n = G * cgs[b] * 128
                    nch = G * cgs[b]
                    off = off_gb[(gi, b)]
                    jg = off // 128
                    m = mp.tile([128, CHUNK_W], b16, tag="mp")
                    done = 0
                    while done < n:
                        step = min(SUBCALL, n - done)
                        nc.gpsimd.dma_gather(
                            out_ap=m[:, done:done + step].rearrange(
                                "p (c d) -> p c d", d=128),
                            in_ap=table[b][:, :],
                            idxs_ap=my_idx[:, (off + done) // 16:
                                           (off + done + step) // 16],
                            num_idxs=step,
                            num_idxs_reg=step,
                            elem_size=128,
                            queue_num=qstate[0] % NQUEUES,
                        )
                        qstate[0] += 1
                        done += step
                    if not packed:
                        oh = ohp.tile([128, CHUNK_W], b16, tag="ohp")
                        nc.vector.tensor_tensor(
                            out=oh[:, :n].rearrange("p (c d) -> p c d", d=128),
                            in0=iota_s[:].unsqueeze(1).broadcast_to(
                                [128, nch, 128]),
                            in1=colr_s[:, jg:jg + nch].unsqueeze(2).broadcast_to(
                                [128, nch, 128]),
                            op=ISEQ,
                        )
                        for ti in range(G):
                            for cc in range(cgs[b]):
                                jj = (ti * cgs[b] + cc) * 128
                                nc.tensor.matmul(
                                    out=pC[:, ti * 64:(ti + 1) * 64],
                                    lhsT=oh[:, jj:jj + 128],
                                    rhs=m[:, jj:jj + 64],
                                    start=(cc == 0),
                                    stop=(cc == cgs[b] - 1),
                                )
                    else:
                        ohA = ohp.tile([128, CHUNK_W], b16, tag="ohp")
                        nc.vector.tensor_tensor(
                            out=ohA[:, :n].rearrange("p (c d) -> p c d", d=128),
                            in0=iota_s[:].unsqueeze(1).broadcast_to(
                                [128, nch, 128]),
                            in1=colrA_s[:, jg:jg + nch].unsqueeze(2).broadcast_to(
                                [128, nch, 128]),
                            op=ISEQ,
                        )
                        ohB = ohp.tile([128, CHUNK_W], b16, tag="ohp")
                        nc.vector.tensor_tensor(
                            out=ohB[:, :n].rearrange("p (c d) -> p c d", d=128),
                            in0=iota_s[:].unsqueeze(1).broadcast_to(
                                [128, nch, 128]),
                            in1=colrB_s[:, jg:jg + nch].unsqueeze(2).broadcast_to(
                                [128, nch, 128]),
                            op=ISEQ,
                        )
                        for ti in range(G):
                            for cc in range(cgs[b]):
                                jj = (ti * cgs[b] + cc) * 128
                                nc.tensor.matmul(
                                    out=pC[:, ti * 64:(ti + 1) * 64],
                                    lhsT=ohA[:, jj:jj + 128],
                                    rhs=m[:, jj:jj + 64],
                                    start=(cc == 0),
                                    stop=False,
                                )
                                nc.tensor.matmul(
                                    out=pC[:, ti * 64:(ti + 1) * 64],
                                    lhsT=ohB[:, jj:jj + 128],
                                    rhs=m[:, jj + 64:jj + 128],
                                    start=False,
                                    stop=(cc == cgs[b] - 1),
                                )
                    if b == 0:
                        nc.vector.tensor_copy(out=acc[:], in_=pC[:])
                    else:
                        nc.vector.tensor_tensor(out=acc[:], in0=acc[:],
                                                in1=pC[:], op=ADD)
                return acc

            groups_of_chunk = []
            _gi = 0
            for c in range(NBLK):
                tiles_left = CHUNK_TILES[c]
                lst = []
                while tiles_left > 0:
                    lst.append(_gi)
                    tiles_left -= groups[_gi][0]
                    _gi += 1
                groups_of_chunk.append(lst)

            # ---------------- C1 + D + chunked AG2 ----------------
            root2 = []
            for c in range(NBLK):
                for gi in groups_of_chunk[c]:
                    G = groups[gi][0]
                    pC = edge_group(h_full, gi)
                    for tp in range(G):
                        t = int(g_starts[gi]) + tp
                        s1 = sp.tile([128, 64], f32, tag="s1")
                        nc.scalar.activation(s1[:], pC[:, tp * 64:(tp + 1) * 64],
                                             Copy, scale=dinv_s[:, t:t + 1])
                        s2 = sp.tile([128, 64], f32, tag="s2")
                        nc.vector.tensor_tensor(out=s2[:], in0=s1[:],
                                                in1=root1[t][:], op=ADD)
                        pT = psR.tile([128, 128], f32, tag="psT")
                        nc.tensor.transpose(out=pT[:64, :], in_=s2[:],
                                            identity=ident_s[:])
                        h1t = h1p.tile([64, 128], b16, tag="h1t")
                        nc.scalar.activation(h1t[:], pT[:64, :], Relu)
                        pDB = psR.tile([128, 128], f32, tag="psDB")
                        nc.tensor.matmul(out=pDB[:, 0:64], lhsT=h1t[:],
                                         rhs=w2i_s[:], start=True, stop=True)
                        nc.tensor.matmul(out=pDB[:, 64:128], lhsT=h1t[:],
                                         rhs=w2r_s[:], start=True, stop=True)
                        hht = hbp.tile([128, 64], b16, tag="hb2")
                        nc.scalar.activation(hht[:], pDB[:, 0:64], Copy,
                                             scale=dinv_s[:, t:t + 1])
                        nc.sync.dma_start(
                            hh_own[t * 64:(t + 1) * 64, 0:64], hht[0:64, :])
                        nc.sync.dma_start(
                            hh_own[t * 64:(t + 1) * 64, 64:128], hht[64:128, :])
                        r2 = rootB.tile([128, 64], b16, tag="rootB")
                        nc.vector.tensor_tensor(out=r2[:], in0=pDB[:, 64:128],
                                                in1=b2_s[:], op=ADD)
                        root2.append(r2)
                r0 = int(CHUNK_START[c]) // 2
                r1_ = r0 + CHUNK_ROWS[c] // 2
                nc.gpsimd.collective_compute(
                    "AllGather", mybir.AluOpType.bypass,
                    replica_groups=[list(range(N_CORES))],
                    ins=[hh_own[r0:r1_, :].opt()], outs=[hh_full[c][:].opt()],
                )

            # ---------------- C2 -> output ----------------
            for gi in range(len(groups)):
                G = groups[gi][0]
                pC = edge_group(hh_full, gi, packed=True)
                for tp in range(G):
                    t = int(g_starts[gi]) + tp
                    s1 = sp.tile([128, 64], f32, tag="s1b")
                    nc.scalar.activation(s1[:], pC[:, tp * 64:(tp + 1) * 64],
                                         Copy, scale=dinv_s[:, t:t + 1])
                    s2 = sp.tile([128, 64], f32, tag="s2b")
                    nc.vector.tensor_tensor(out=s2[:], in0=s1[:],
                                            in1=root2[t][:], op=ADD)
                    o = op_.tile([128, 64], f32, tag="o")
                    nc.scalar.activation(o[:], s2[:], Relu)
                    nc.sync.dma_start(t_out[t * 128:(t + 1) * 128, :], o[:])

    nc.compile()
    return nc


# ----------------------------------------------------------------------------
# entry point
# ----------------------------------------------------------------------------

_LAST_RESULTS = None


def build_in_maps(inputs, streams):
    x = np.asarray(inputs["x"], np.float32)
    iota = np.broadcast_to(np.arange(128, dtype=np.float32), (128, 128)).astype(bf16)
    ident = np.eye(128, dtype=np.float32)
    w1i = np.asarray(inputs["w1_init"], np.float32)
    w1r = np.asarray(inputs["w1_root"], np.float32)
    w2i = np.zeros((64, 64), np.float32); w2i[:, :CLS] = inputs["w2_init"]
    w2r = np.zeros((64, 64), np.float32); w2r[:, :CLS] = inputs["w2_root"]
    b1b = np.broadcast_to(np.asarray(inputs["b1"], np.float32), (128, HID)).copy()
    b2p = np.zeros(64, np.float32); b2p[:CLS] = inputs["b2"]
    b2b = np.broadcast_to(b2p, (128, 64)).copy()
    xs_all = np.zeros((VPAD, 128), np.float32)
    col = 0
    for c in range(NBLK):
        for k in range(N_CORES):
            s0 = int(CHUNK_START[c]); nrow = CHUNK_ROWS[c]
            slots = np.arange(s0, s0 + nrow)
            perm = streams[k]["perm"]
            nodes = perm[slots]
            v = nodes >= 0
            dv = streams[k]["dinvT"].T.reshape(-1)
            blk = np.zeros((nrow, 128), np.float32)
            blk[v] = x[nodes[v]] * dv[slots[v], None]
            xs_all[col:col + nrow] = blk
            col += nrow
    xsb = xs_all.T.astype(bf16).copy()
    in_maps = []
    for k in range(N_CORES):
        s = streams[k]
        perm = s["perm"]
        xk = np.zeros((SHARD_PAD, 128), np.float32)
        v = perm >= 0
        xk[v] = x[perm[v]]
        in_maps.append(dict(
            xT=xk.T.astype(bf16).copy(),
            xsb=xsb,
            idx=s["idx"], idx2=s["idx2"], colr=np.asarray(s["colr"]),
            colrA=np.asarray(s["colrA"]), colrB=np.asarray(s["colrB"]),
            dinvT=s["dinvT"],
            iota=np.asarray(iota), ident=ident,
            w1i=w1i.astype(bf16), w1r=w1r.astype(bf16),
            w2i=w2i.astype(bf16), w2r=w2r.astype(bf16),
            b1b=b1b, b2b=b2b,
        ))
    return in_maps


def kernel(x, edge_index, w1_init, w1_root, b1, w2_init, w2_root, b2, **kw):
    global _LAST_RESULTS
    from concourse.bass_utils import run_bass_kernel_spmd

    inputs = dict(x=x, edge_index=edge_index, w1_init=w1_init, w1_root=w1_root,
                  b1=b1, w2_init=w2_init, w2_root=w2_root, b2=b2)
    streams, groups, TOT = _prep(np.asarray(edge_index))

    key = (groups, TOT)
    if key not in _PROG_CACHE:
        _PROG_CACHE[key] = _build_program(groups, TOT)
    nc = _PROG_CACHE[key]

    in_maps = build_in_maps(inputs, streams)

    import os
    trace = os.environ.get("BIARMA_TRACE", "0") == "1"
    res = run_bass_kernel_spmd(nc, in_maps, core_ids=list(range(N_CORES)),
                               trace=trace)
    _LAST_RESULTS = res

    out = np.zeros((N_NODES, CLS), np.float32)
    for k in range(N_CORES):
        o = res.results[k]["out"]
        perm = streams[k]["perm"]
        v = perm >= 0
        out[perm[v]] = o[v][:, :CLS]
    return out

